# revision 7
# baseline (speedup 1.0000x reference)
"""Trainium2 Bass kernel: multi-head causal self-attention with RoPE.

Computes, for x:[B,S,D], Wq/Wk/Wv/Wo:[D,D] (B=2, S=2048, D=1024, H=16 heads,
hd=64):
    q/k/v = (x @ W{q,k,v}.T) -> [B,H,S,hd];  q,k = rope(q), rope(k)
    out   = softmax(causal(q k^T / sqrt(hd))) v   -> merge heads -> @ Wo.T

Sharding: 8 NeuronCores = (2 batches) x (4 head-groups of 4 heads).  Each
core computes its 4 heads' attention plus the partial output projection
(columns of Wo belonging to its heads); the host sums the 4 partial outputs
per batch.

Per-core dataflow (everything in "transposed" space so no PE transposes are
needed):
    xT [D,S] -> QT,KT [hd,S] per head (fp32r matmuls) -> RoPE (DVE shuffle
    + mul/add split across DVE and Pool) -> scoresT[k,q] = KT^T-slice
    matmuls -> exp on ACT (no max-subtraction: |scores/8| <= ~3.2) -> PV
    with a ones-column appended to V so row 64 of the accumulator is the
    softmax denominator -> normalize -> output projection from the
    transposed head outputs.

Scheduling: the emission order software-pipelines the PE stream.  Within
attention for q-chunk qc, PV for kt-pair j is emitted one iteration late
(while exp(j+1) runs on ACT), and QKV matmul groups for chunk qc+1 plus the
O-projection for chunk qc-1 are interleaved between attention pairs so the
PE never sits idle waiting on ACT.  Causal masks are 256-wide fp32r matmuls
(128-wide ones pay a 4x penalty) using two constants: triA=[tri|0] for
even-coff tiles, triB=[-400|tri] for odd-coff tiles; the -400 block also
zeroes (post-exp) the columns that widened PV matmuls read.
"""

import sys

sys.path.insert(0, "/opt/trn_rl_repo")

import numpy as np

import concourse.bass as bass
import concourse.mybir as mybir
import concourse.tile as tile
from concourse.bass_utils import run_bass_kernel_spmd

F32 = mybir.dt.float32
F32R = mybir.dt.float32r
AF = mybir.ActivationFunctionType
OP = mybir.AluOpType

# stream_shuffle's 32-entry mask is a per-quadrant partition permutation
# (applied identically to all four 32-partition quadrants).  We therefore
# store head dims interleaved -- partition 64h+2i holds dim i, 64h+2i+1
# holds dim 32+i -- so the RoPE pair swap is an adjacent-pair exchange.
# The interleave is a shared permutation of Q and K dims (folded into the
# weight slices and rope tables on the host), which leaves q.k scores
# unchanged.
SWAP_MASK = [i ^ 1 for i in range(32)]

HD = 64
HALF = HD // 2
ROPE_BASE = 10000.0


def _split_waits(nc, maxw=1):
    """walrus in this container rejects instructions with more than a couple
    of semaphore waits; hoist excess waits onto preceding NoOps."""
    ctr = 0
    for bb in nc.main_func.blocks:
        insts = bb.instructions
        new = []
        changed = False
        for ins in insts:
            si = ins.sync_info
            if si is not None and si.on_wait and len(si.on_wait) > maxw:
                waits = list(si.on_wait)
                keep, rest = waits[:maxw], waits[maxw:]
                for i in range(0, len(rest), maxw):
                    ctr += 1
                    new.append(mybir.InstNoOp(
                        name=f"WSPLIT-{ctr}", opcode="NoOp", engine=ins.engine,
                        sync_info=mybir.SyncInfo(on_wait=rest[i:i + maxw], on_update=[])))
                si.on_wait = keep
                changed = True
            new.append(ins)
        if changed:
            bb.instructions = new


def build_program(S, D, HPC=4, repeat=1, use_loop=False, phase=4):
    """One-core SPMD program: attention for HPC heads of one batch.

    phase (for perf bisection): -1 = input DMA only, 0 = +QKV matmuls,
    1 = +rope/V-evac, 2 = +scores/exp/PV, 3 = +normalize, 4 = full.
    use_loop wraps `repeat` copies of the body in a tc.For_i (constant
    NEFF size -- used by the loop-slope timing harness).
    """
    NKT = D // 128          # k-tiles over the embedding dim
    NSC = S // 512          # 512-wide s-chunks
    NST = S // 128          # 128-wide s-tiles
    G = HPC // 2            # head pairs
    E = HPC * HD            # per-core head dims
    VW = HD + 1

    nc = bass.Bass()
    # pre-tiled host layouts: xT[p, sc*NKT*512 + kt*512 + s'] = x[sc*512+s', kt*128+p]
    # wq/wk are head-pair-major: col g*(NKT*128) + kt*128 + c
    xT = nc.declare_dram_parameter("xT", [128, S * NKT], F32R, isOutput=False)
    wq = nc.declare_dram_parameter("wq", [128, G * NKT * 128], F32R, isOutput=False)
    wk = nc.declare_dram_parameter("wk", [128, G * NKT * 128], F32R, isOutput=False)
    wv = nc.declare_dram_parameter("wv", [128, NKT * E], F32R, isOutput=False)
    wo = nc.declare_dram_parameter("wo", [128, G * D], F32R, isOutput=False)
    cs = nc.declare_dram_parameter("cs", [128, S], F32, isOutput=False)
    sn = nc.declare_dram_parameter("sn", [128, S], F32, isOutput=False)
    triA = nc.declare_dram_parameter("triA", [128, 256], F32R, isOutput=False)
    triB = nc.declare_dram_parameter("triB", [128, 256], F32R, isOutput=False)
    idn = nc.declare_dram_parameter("idn", [128, 128], F32R, isOutput=False)
    out = nc.declare_dram_parameter("out", [S, D], F32, isOutput=True)

    with tile.TileContext(nc) as tc, \
         nc.allow_low_precision(reason="float32r operands feed the PE at full rate"):
        with (tc.tile_pool(name="wp", bufs=1) as wp,
              tc.tile_pool(name="xp", bufs=2) as xp,
              tc.tile_pool(name="rt", bufs=2) as rt,
              tc.tile_pool(name="ptp", bufs=3) as ptp,
              tc.tile_pool(name="rcp", bufs=2) as rcp,
              tc.tile_pool(name="oevp", bufs=2) as oevp,
              tc.tile_pool(name="ps", bufs=1, space="PSUM") as ps_pool,
              tc.tile_pool(name="ot_ps", bufs=2, space="PSUM") as ot_ps):

            # ---- persistent tiles
            wq_s = wp.tile([128, G * NKT * 128], F32R, name="wq_s")
            wk_s = wp.tile([128, G * NKT * 128], F32R, name="wk_s")
            wv_s = wp.tile([128, NKT * E], F32R, name="wv_s")
            wo_s = wp.tile([128, G * D], F32R, name="wo_s")
            cs_s = wp.tile([128, S], F32, name="cs_s")
            sn_s = wp.tile([128, S], F32, name="sn_s")
            triA_s = wp.tile([128, 256], F32R, name="triA_s")
            triB_s = wp.tile([128, 256], F32R, name="triB_s")
            on_s = wp.tile([128, 64], F32R, name="on_s")
            idn_s = wp.tile([128, 128], F32R, name="idn_s")
            qtr = wp.tile([128, G * S], F32R, name="qtr")
            ktr = wp.tile([128, G * S], F32R, name="ktr")
            vv = wp.tile([128, NST * HPC * VW], F32R, name="vv")
            otn = wp.tile([128, G * S], F32R, name="otn")

            # weight/const DMAs, ordered so the first QKV group's operands
            # land first (per-g slices of wq/wk are contiguous)
            HW = NKT * 128  # columns per head-pair in wq/wk
            nc.sync.dma_start(wq_s[:, 0:HW], wq[:, 0:HW])
            nc.sync.dma_start(wk_s[:, 0:HW], wk[:, 0:HW])
            nc.sync.dma_start(cs_s[:, 0:S // 2], cs[:, 0:S // 2])
            nc.sync.dma_start(sn_s[:, 0:S // 2], sn[:, 0:S // 2])
            nc.sync.dma_start(wq_s[:, HW:2 * HW], wq[:, HW:2 * HW])
            nc.sync.dma_start(wk_s[:, HW:2 * HW], wk[:, HW:2 * HW])
            nc.sync.dma_start(wv_s[:], wv[:])
            nc.sync.dma_start(triA_s[:], triA[:])
            nc.sync.dma_start(triB_s[:], triB[:])
            nc.sync.dma_start(idn_s[:], idn[:])
            nc.sync.dma_start(cs_s[:, S // 2:S], cs[:, S // 2:S])
            nc.sync.dma_start(sn_s[:, S // 2:S], sn[:, S // 2:S])
            nc.sync.dma_start(wo_s[:], wo[:])
            # ones columns of V+ (for the softmax denominator)
            vv_r = vv[:].rearrange("p (st h c) -> p st h c", st=NST, h=HPC, c=VW)
            nc.vector.memset(vv_r[:, :, :, HD:HD + 1].bitcast(F32), 1.0)
            nc.vector.memset(on_s[:].bitcast(F32), 1.0)

            def body():
                xtas = {}

                # ================= QKV + RoPE work items =================
                def make_qkv_items(sc, vg_defer=0):
                    def fetch():
                        # one 2MB transfer: ~300 GB/s vs ~90 GB/s for 512KB
                        # slices (per-dma fixed cost dominates small transfers)
                        xta = xp.tile([128, NKT * 512], F32R, tag="x", name="xta")
                        nc.sync.dma_start(
                            xta[:], xT[:, sc * NKT * 512:(sc + 1) * NKT * 512])
                        xtas[sc] = xta

                    def qk(w_s, g, dst):
                        def f():
                            xta = xtas[sc]
                            if phase < 0:
                                return
                            ps = ps_pool.tile([128, 512], F32, tag="mm", bufs=2, name="ps")
                            for kt in range(NKT):
                                nc.tensor.matmul(
                                    ps[:], w_s[:, (g * NKT + kt) * 128:(g * NKT + kt + 1) * 128],
                                    xta[:, kt * 512:(kt + 1) * 512],
                                    start=(kt == 0), stop=(kt == NKT - 1))
                            if phase == 0:
                                return
                            # RoPE: rot = ps*cos + swap(ps)*sgn_sin
                            qsw = rt.tile([128, 512], F32, tag="qsw", name="qsw")
                            m1 = rt.tile([128, 512], F32, tag="m1", name="m1")
                            m2 = rt.tile([128, 512], F32, tag="m2", name="m2")
                            nc.vector.stream_shuffle(qsw[:], ps[:], SWAP_MASK)
                            nc.vector.tensor_tensor(m1[:], ps[:], cs_s[:, sc * 512:(sc + 1) * 512], OP.mult)
                            nc.gpsimd.tensor_tensor(m2[:], qsw[:], sn_s[:, sc * 512:(sc + 1) * 512], OP.mult)
                            nc.gpsimd.tensor_tensor(
                                dst[:, g * S + sc * 512: g * S + (sc + 1) * 512], m1[:], m2[:], OP.add)
                        return f

                    def vg(stl):
                        def f():
                            xta = xtas[sc]
                            if phase < 0:
                                return
                            st = sc * 4 + stl
                            ps = ps_pool.tile([128, 512], F32, tag="mm", bufs=2, name="psv")
                            for kt in range(NKT):
                                nc.tensor.matmul(
                                    ps[:, 0:E], xta[:, kt * 512 + stl * 128: kt * 512 + (stl + 1) * 128],
                                    wv_s[:, kt * E:(kt + 1) * E], start=(kt == 0), stop=(kt == NKT - 1))
                            if phase == 0:
                                return
                            nc.vector.tensor_copy(
                                vv_r[:, st, :, 0:HD], ps[:, 0:E].rearrange("p (h c) -> p h c", h=HPC, c=HD))
                        return f

                    items = [fetch,
                             qk(wq_s, 0, qtr), qk(wk_s, 0, ktr),
                             qk(wq_s, 1, qtr), qk(wk_s, 1, ktr)]
                    items += [vg(stl) for stl in range(4)]
                    if vg_defer:
                        return items[:-vg_defer], items[-vg_defer:]
                    return items, []

                # ================= attention work items =================
                # Per (q-chunk, head-pair): the two heads' K=64 score matmuls
                # sit in different PE row groups (partition bases 0/64) and
                # overlap in the array.  k-tiles are processed two at a time
                # so one exp covers a [128,1024] two-bank PSUM tile.  PV for
                # a pair is emitted one iteration later so its exp is hidden
                # behind the next pair's score matmuls.
                def cstart_of(kt, qc):
                    coff = max(0, 128 * kt - 512 * qc)
                    return min(coff, 256), coff

                def make_attn_g_items(qc, g):
                    nkt_q = 4 * qc + 4
                    st8 = {}
                    prev = [None]
                    pairs = list(range(0, nkt_q, 2))

                    def emit_pv():
                        ptA, ptB, kts = prev[0]
                        ots = st8["ot"]
                        for j, kt in enumerate(kts):
                            cst, _ = cstart_of(kt, qc)
                            for hh, pt, ot in ((2 * g, ptA, ots[0]), (2 * g + 1, ptB, ots[1])):
                                nc.tensor.matmul(
                                    ot[0:VW, cst:512],
                                    vv[:, kt * HPC * VW + hh * VW: kt * HPC * VW + (hh + 1) * VW],
                                    pt[:, j * 512 + cst: (j + 1) * 512],
                                    start=(kt == 0), stop=(kt == nkt_q - 1))

                    def mk_pair(kt2, first):
                        def f():
                            if first:
                                st8["ot"] = (ot_ps.tile([128, 512], F32, tag="ot", name="otA"),
                                             ot_ps.tile([128, 512], F32, tag="ot", name="otB"))
                            kts = [kt2] + ([kt2 + 1] if kt2 + 1 < nkt_q else [])
                            sA = ps_pool.tile([128, 1024], F32, tag="sc2", bufs=2, name="sA")
                            sB = ps_pool.tile([128, 1024], F32, tag="sc2", bufs=2, name="sB")
                            for j, kt in enumerate(kts):
                                cst, coff = cstart_of(kt, qc)
                                diag = kt >= 4 * qc
                                for hp, stile in ((0, sA), (64, sB)):
                                    nc.tensor.matmul(
                                        stile[:, j * 512 + cst: (j + 1) * 512],
                                        ktr[hp:hp + 64, g * S + kt * 128: g * S + (kt + 1) * 128],
                                        qtr[hp:hp + 64, g * S + qc * 512 + cst: g * S + (qc + 1) * 512],
                                        start=True, stop=not diag)
                                    if diag:
                                        # additive causal mask, 256 wide:
                                        # even coff: [tri|0] at coff,
                                        # odd coff: [-400|tri] at coff-128
                                        tri_s, m0 = (triA_s, coff) if coff % 256 == 0 \
                                            else (triB_s, coff - 128)
                                        nc.tensor.matmul(
                                            stile[:, j * 512 + m0: j * 512 + m0 + 256],
                                            idn_s[:], tri_s[:], start=False, stop=True)
                            c0, _ = cstart_of(kts[0], qc)
                            cend = len(kts) * 512
                            ptA = ptp.tile([128, 1024], F32R, tag="pt", name="ptA")
                            ptB = ptp.tile([128, 1024], F32R, tag="pt", name="ptB")
                            nc.scalar.activation(ptA[:, c0:cend], sA[:, c0:cend], AF.Exp, scale=0.125)
                            nc.scalar.activation(ptB[:, c0:cend], sB[:, c0:cend], AF.Exp, scale=0.125)
                            if prev[0] is not None:
                                emit_pv()
                            prev[0] = (ptA, ptB, kts)
                        return f

                    def tail():
                        otA, otB = st8["ot"]
                        emit_pv()
                        if phase < 3:
                            return
                        # normalize by the denominator row: reciprocal of row
                        # 64, broadcast across 64 partitions via a K=1 matmul,
                        # then multiply out of PSUM.
                        rcA = rcp.tile([128, 512], F32R, tag="rcA", name="rcA")
                        rcB = rcp.tile([128, 512], F32R, tag="rcB", name="rcB")
                        nc.vector.reciprocal(rcA[HD:HD + 1, :], otA[HD:HD + 1, :])
                        nc.vector.reciprocal(rcB[HD:HD + 1, :], otB[HD:HD + 1, :])
                        rbA = ps_pool.tile([128, 512], F32, tag="mm", bufs=2, name="rbA")
                        rbB = ps_pool.tile([128, 512], F32, tag="mm", bufs=2, name="rbB")
                        nc.tensor.matmul(rbA[0:HD, :], on_s[HD:HD + 1, 0:HD],
                                         rcA[HD:HD + 1, :], start=True, stop=True)
                        nc.tensor.matmul(rbB[0:HD, :], on_s[HD:HD + 1, 0:HD],
                                         rcB[HD:HD + 1, :], start=True, stop=True)
                        nc.vector.tensor_copy(rcA[0:HD, :], rbA[0:HD, :])
                        nc.vector.tensor_copy(rcB[0:HD, :], rbB[0:HD, :])
                        for hp, ot, rc in ((0, otA, rcA), (64, otB, rcB)):
                            nc.vector.tensor_tensor(
                                otn[hp:hp + 64, g * S + qc * 512: g * S + (qc + 1) * 512],
                                ot[0:HD, :], rc[0:HD, :], OP.mult)

                    items = [mk_pair(kt2, idx == 0) for idx, kt2 in enumerate(pairs)]
                    items.append(tail)
                    return items

                def make_attn_items(qc):
                    if phase < 2:
                        return []
                    items = []
                    for g in range(G):
                        items += make_attn_g_items(qc, g)
                    return items

                # ================= output projection work items =================
                def make_oproj_items(qc):
                    if phase < 4:
                        return []
                    CW = min(512, D)
                    NPAIR = 2 if S >= 512 else 1
                    items = []

                    def mk(st2):
                        def f():
                            oev = oevp.tile([128, NPAIR * D], F32, tag="oev", name="oev")
                            for half in range(NPAIR):
                                st = qc * 4 + st2 * NPAIR + half
                                for nch in range(D // CW):
                                    op = ps_pool.tile([128, 512], F32, tag="mm", bufs=2, name="opps")
                                    for g in range(G):
                                        nc.tensor.matmul(
                                            op[:, 0:CW], otn[:, g * S + st * 128:g * S + (st + 1) * 128],
                                            wo_s[:, g * D + nch * CW: g * D + (nch + 1) * CW],
                                            start=(g == 0), stop=(g == G - 1))
                                    if nch % 2 == 0:
                                        nc.scalar.copy(
                                            oev[:, half * D + nch * CW: half * D + (nch + 1) * CW], op[:, 0:CW])
                                    else:
                                        nc.vector.tensor_copy(
                                            oev[:, half * D + nch * CW: half * D + (nch + 1) * CW], op[:, 0:CW])
                            st0 = qc * 4 + st2 * NPAIR
                            dst = out[st0 * 128:(st0 + NPAIR) * 128, :].rearrange(
                                "(b p) d -> p b d", b=NPAIR)
                            # ACT's HWDGE ring: keeps the SP ring free for the
                            # x fetches (two parallel DMA FIFOs)
                            nc.scalar.dma_start(dst, oev[:].rearrange("p (b d) -> p b d", b=NPAIR))
                        return f

                    return [mk(st2) for st2 in range(4 // NPAIR)]

                def merge(a, b):
                    out_items = []
                    j = 0
                    for i, x in enumerate(a):
                        out_items.append(x)
                        want = (i + 1) * len(b) // len(a) if a else len(b)
                        while j < want:
                            out_items.append(b[j])
                            j += 1
                    out_items.extend(b[j:])
                    return out_items

                # drive: chunk 0's QKV runs alone; attention for chunk qc is
                # interleaved with QKV for chunk qc+1 and the O-projection
                # for chunk qc-1.  Two V-groups of the last chunk and the
                # last O-projections pad the final (ACT-heaviest) phase.
                sched = []
                it0, _ = make_qkv_items(0)
                sched += it0
                deferred = []
                for sc in range(1, NSC):
                    fill, d2 = make_qkv_items(sc, vg_defer=2 if sc == NSC - 1 else 0)
                    if d2:
                        deferred = d2
                    if sc >= 2:
                        fill = fill + make_oproj_items(sc - 2)
                    sched += merge(make_attn_items(sc - 1), fill)
                # deferred V-groups must land before the PV matmuls that read
                # them (program order defines dataflow): spread them over the
                # first pairs of the last attention phase only.
                last_attn = make_attn_items(NSC - 1)
                npair0 = (4 * (NSC - 1) + 4) // 2   # pairs in g=0
                sched += merge(last_attn[:npair0 - 1], deferred)
                sched += merge(last_attn[npair0 - 1:],
                               make_oproj_items(NSC - 2) if NSC >= 2 else [])
                sched += make_oproj_items(NSC - 1)
                for it in sched:
                    it()

            if use_loop:
                with tc.For_i(0, repeat, 1):
                    body()
            else:
                for _ in range(repeat):
                    body()

    _split_waits(nc)
    return nc


def _rope_tables(S):
    # interleaved dim order: within each 64-partition head block, partition
    # j=2i holds dim i (gets cos, -sin), j=2i+1 holds dim 32+i (cos, +sin)
    inv = 1.0 / (ROPE_BASE ** (np.arange(HALF, dtype=np.float64) / HALF))
    ang = np.arange(S, dtype=np.float64)[:, None] * inv[None, :]  # [S, HALF]
    cos, sin = np.cos(ang), np.sin(ang)
    j = np.arange(128) % HD
    freq = j // 2
    cs = cos[:, freq].T.astype(np.float32)                # [128, S]
    sgn = np.where(j % 2 == 0, -1.0, 1.0)
    sn = (sin[:, freq] * sgn[None, :]).T.astype(np.float32)
    return np.ascontiguousarray(cs), np.ascontiguousarray(sn)


def _tile_rows(a, nt):
    """[nt*128, C] -> [128, nt*C] with block kt at cols [kt*C, (kt+1)*C)."""
    n, c = a.shape
    assert n == nt * 128
    return np.ascontiguousarray(a.reshape(nt, 128, c).transpose(1, 0, 2).reshape(128, nt * c))


def _tile_rows_g(a, nt, g):
    """[nt*128, g*128] -> [128, g*nt*128], head-pair-major: col
    gi*(nt*128) + kt*128 + c = a[kt*128 + p, gi*128 + c]."""
    n, c = a.shape
    assert n == nt * 128 and c == g * 128
    return np.ascontiguousarray(
        a.reshape(nt, 128, g, 128).transpose(1, 2, 0, 3).reshape(128, g * nt * 128))


def _prep_x(x_b, D, S):
    """[S, D] -> [128, S*NKT]: col sc*(NKT*512) + kt*512 + s' = x[sc*512+s', kt*128+p]."""
    NKT, NSC = D // 128, S // 512
    return np.ascontiguousarray(
        x_b.reshape(NSC, 512, NKT, 128).transpose(3, 0, 2, 1).reshape(128, S * NKT))


def _mask_consts():
    # additive causal masks in [k, q] layout (0 where k <= q, else -400;
    # -50 after the 1/8 softmax scale -> exp underflows to ~2e-22).
    # triA = [tri | zeros] applied at even coff; triB = [-400 | tri]
    # applied 128 left of odd coff (the -400 block covers the fully-masked
    # zone that widened PV matmuls read through).
    tri = np.where(np.triu(np.ones((128, 128), dtype=bool)), 0.0, -400.0).astype(np.float32)
    zeros = np.zeros((128, 128), dtype=np.float32)
    neg = np.full((128, 128), -400.0, dtype=np.float32)
    triA = np.ascontiguousarray(np.concatenate([tri, zeros], axis=1))
    triB = np.ascontiguousarray(np.concatenate([neg, tri], axis=1))
    idn = np.eye(128, dtype=np.float32)
    return triA, triB, idn


def _interleave_perm(n_heads):
    """Permutation of head-dim rows: new row 64h+2i <- old 64h+i,
    new row 64h+2i+1 <- old 64h+32+i."""
    perm = np.empty(n_heads * HD, dtype=np.int64)
    for h in range(n_heads):
        base = h * HD
        for i in range(HALF):
            perm[base + 2 * i] = base + i
            perm[base + 2 * i + 1] = base + HALF + i
    return perm


_PROG_CACHE = {}


def kernel(x, Wq, Wk, Wv, Wo):
    B, S, D = x.shape
    H = 16
    HPC = 4                      # heads per core
    GROUPS = H // HPC            # 4 head-groups
    N_CORES = B * GROUPS
    G = HPC // 2

    x = np.asarray(x, dtype=np.float32)
    Wq, Wk, Wv, Wo = (np.asarray(w, dtype=np.float32) for w in (Wq, Wk, Wv, Wo))

    cs, sn = _rope_tables(S)
    triA, triB, idn = _mask_consts()
    NKT = D // 128
    xTs = [_prep_x(x[b], D, S) for b in range(B)]

    perm = _interleave_perm(HPC)
    in_maps = []
    for c in range(N_CORES):
        b, hg = divmod(c, GROUPS)
        e0 = hg * HPC * HD
        e1 = e0 + HPC * HD
        in_maps.append({
            "xT": xTs[b],
            "wq": _tile_rows_g(Wq[e0:e1, :][perm].T, NKT, G),
            "wk": _tile_rows_g(Wk[e0:e1, :][perm].T, NKT, G),
            "wv": _tile_rows(Wv[e0:e1, :].T, NKT),
            "wo": _tile_rows(Wo[:, e0:e1].T, 2),
            "cs": cs, "sn": sn, "triA": triA, "triB": triB, "idn": idn,
        })

    key = (S, D, HPC)
    if key not in _PROG_CACHE:
        _PROG_CACHE[key] = build_program(S, D, HPC)
    nc = _PROG_CACHE[key]
    res = run_bass_kernel_spmd(nc, in_maps, list(range(N_CORES)))

    out = np.zeros((B, S, D), dtype=np.float64)
    for c in range(N_CORES):
        b = c // GROUPS
        out[b] += res.results[c]["out"].astype(np.float64)
    return out.astype(np.float32)


if __name__ == "__main__":
    # mini self-test: one core, small S/D, against a numpy model
    S, D, HPC = 512, 256, 4
    G = HPC // 2
    rng = np.random.default_rng(0)
    x = rng.standard_normal((S, D)).astype(np.float32)
    bound = 1.0 / np.sqrt(D)
    Wq, Wk, Wv = (rng.uniform(-bound, bound, (HPC * HD, D)).astype(np.float32) for _ in range(3))
    Wo = rng.uniform(-bound, bound, (D, HPC * HD)).astype(np.float32)

    # numpy reference (same math as reference.py, restricted to HPC heads)
    q = (x @ Wq.T).reshape(S, HPC, HD).transpose(1, 0, 2)
    k = (x @ Wk.T).reshape(S, HPC, HD).transpose(1, 0, 2)
    v = (x @ Wv.T).reshape(S, HPC, HD).transpose(1, 0, 2)
    inv = 1.0 / (ROPE_BASE ** (np.arange(HALF) / HALF))
    ang = np.arange(S)[:, None] * inv[None, :]
    cosr, sinr = np.cos(ang), np.sin(ang)

    def rope(t):
        t1, t2 = t[..., :HALF], t[..., HALF:]
        return np.concatenate([t1 * cosr - t2 * sinr, t1 * sinr + t2 * cosr], -1)

    q, k = rope(q), rope(k)
    sc_ = np.einsum("hqd,hkd->hqk", q, k) / np.sqrt(HD)
    mask = np.tril(np.ones((S, S), dtype=bool))
    sc_ = np.where(mask, sc_, -np.inf)
    p = np.exp(sc_ - sc_.max(-1, keepdims=True))
    p /= p.sum(-1, keepdims=True)
    ref = np.einsum("hqk,hkd->hqd", p, v).transpose(1, 0, 2).reshape(S, HPC * HD) @ Wo.T

    cs, sn = _rope_tables(S)
    triA, triB, idn = _mask_consts()
    perm = _interleave_perm(HPC)
    in_map = {
        "xT": _prep_x(x, D, S),
        "wq": _tile_rows_g(Wq[perm].T, D // 128, G),
        "wk": _tile_rows_g(Wk[perm].T, D // 128, G),
        "wv": _tile_rows(Wv.T, D // 128),
        "wo": _tile_rows(Wo.T, 2),
        "cs": cs, "sn": sn, "triA": triA, "triB": triB, "idn": idn,
    }
    nc = build_program(S, D, HPC)
    res = run_bass_kernel_spmd(nc, [in_map], [0])
    got = res.results[0]["out"]
    err = np.abs(got - ref)
    rel = err.max() / np.abs(ref).max()
    rms = np.sqrt((err ** 2).mean()) / np.sqrt((ref ** 2).mean())
    print(f"mini: max abs err {err.max():.3e}  max rel {rel:.3e}  rms rel {rms:.3e}")


# revision 9
# speedup vs baseline: 1.1090x; 1.1090x over previous
"""Trainium2 Bass kernel: multi-head causal self-attention with RoPE.

Computes, for x:[B,S,D], Wq/Wk/Wv/Wo:[D,D] (B=2, S=2048, D=1024, H=16 heads,
hd=64):
    q/k/v = (x @ W{q,k,v}.T) -> [B,H,S,hd];  q,k = rope(q), rope(k)
    out   = softmax(causal(q k^T / sqrt(hd))) v   -> merge heads -> @ Wo.T

Sharding: 8 NeuronCores = (2 batches) x (4 head-groups of 4 heads).  Each
core computes its 4 heads' attention plus the partial output projection
(columns of Wo belonging to its heads); the host sums the 4 partial outputs
per batch.

Per-core dataflow (everything in "transposed" space so no PE transposes are
needed):
    xT [D,S] -> QT,KT [hd,S] per head (fp32r matmuls) -> RoPE (DVE shuffle
    + mul/add split across DVE and Pool) -> scoresT[k,q] = KT^T-slice
    matmuls -> exp on ACT (no max-subtraction: |scores/8| <= ~3.2) -> PV
    with a ones-column appended to V so row 64 of the accumulator is the
    softmax denominator -> normalize -> output projection from the
    transposed head outputs.

Scheduling: the emission order software-pipelines the PE stream.  Within
attention for q-chunk qc, PV for kt-pair j is emitted one iteration late
(while exp(j+1) runs on ACT), and QKV matmul groups for chunk qc+1 plus the
O-projection for chunk qc-1 are interleaved between attention pairs so the
PE never sits idle waiting on ACT.  Causal masks are 256-wide fp32r matmuls
(128-wide ones pay a 4x penalty) using two constants: triA=[tri|0] for
even-coff tiles, triB=[-400|tri] for odd-coff tiles; the -400 block also
zeroes (post-exp) the columns that widened PV matmuls read.
"""

import sys

sys.path.insert(0, "/opt/trn_rl_repo")

import numpy as np

import concourse.bass as bass
import concourse.mybir as mybir
import concourse.tile as tile
from concourse.bass_utils import run_bass_kernel_spmd

F32 = mybir.dt.float32
F32R = mybir.dt.float32r
AF = mybir.ActivationFunctionType
OP = mybir.AluOpType

# stream_shuffle's 32-entry mask is a per-quadrant partition permutation
# (applied identically to all four 32-partition quadrants).  We therefore
# store head dims interleaved -- partition 64h+2i holds dim i, 64h+2i+1
# holds dim 32+i -- so the RoPE pair swap is an adjacent-pair exchange.
# The interleave is a shared permutation of Q and K dims (folded into the
# weight slices and rope tables on the host), which leaves q.k scores
# unchanged.
SWAP_MASK = [i ^ 1 for i in range(32)]

HD = 64
HALF = HD // 2
ROPE_BASE = 10000.0


def _split_waits(nc, maxw=1):
    """walrus in this container rejects instructions with more than a couple
    of semaphore waits; hoist excess waits onto preceding NoOps."""
    ctr = 0
    for bb in nc.main_func.blocks:
        insts = bb.instructions
        new = []
        changed = False
        for ins in insts:
            si = ins.sync_info
            if si is not None and si.on_wait and len(si.on_wait) > maxw:
                waits = list(si.on_wait)
                keep, rest = waits[:maxw], waits[maxw:]
                for i in range(0, len(rest), maxw):
                    ctr += 1
                    new.append(mybir.InstNoOp(
                        name=f"WSPLIT-{ctr}", opcode="NoOp", engine=ins.engine,
                        sync_info=mybir.SyncInfo(on_wait=rest[i:i + maxw], on_update=[])))
                si.on_wait = keep
                changed = True
            new.append(ins)
        if changed:
            bb.instructions = new


def build_program(S, D, HPC=4, repeat=1, use_loop=False, phase=4):
    """One-core SPMD program: attention for HPC heads of one batch.

    phase (for perf bisection): -1 = input DMA only, 0 = +QKV matmuls,
    1 = +rope/V-evac, 2 = +scores/exp/PV, 3 = +normalize, 4 = full.
    use_loop wraps `repeat` copies of the body in a tc.For_i (constant
    NEFF size -- used by the loop-slope timing harness).
    """
    NKT = D // 128          # k-tiles over the embedding dim
    NSC = S // 512          # 512-wide s-chunks
    NST = S // 128          # 128-wide s-tiles
    G = HPC // 2            # head pairs
    E = HPC * HD            # per-core head dims
    VW = HD + 1

    nc = bass.Bass()
    # pre-tiled host layouts: xT[p, sc*NKT*512 + kt*512 + s'] = x[sc*512+s', kt*128+p]
    # wq/wk are head-pair-major: col g*(NKT*128) + kt*128 + c
    xT = nc.declare_dram_parameter("xT", [128, S * NKT], F32R, isOutput=False)
    wq = nc.declare_dram_parameter("wq", [128, G * NKT * 128], F32R, isOutput=False)
    wk = nc.declare_dram_parameter("wk", [128, G * NKT * 128], F32R, isOutput=False)
    wv = nc.declare_dram_parameter("wv", [128, NKT * E], F32R, isOutput=False)
    wo = nc.declare_dram_parameter("wo", [128, G * D], F32R, isOutput=False)
    cs = nc.declare_dram_parameter("cs", [128, S], F32, isOutput=False)
    sn = nc.declare_dram_parameter("sn", [128, S], F32, isOutput=False)
    triA = nc.declare_dram_parameter("triA", [128, 256], F32R, isOutput=False)
    triB = nc.declare_dram_parameter("triB", [128, 256], F32R, isOutput=False)
    idn = nc.declare_dram_parameter("idn", [128, 128], F32R, isOutput=False)
    out = nc.declare_dram_parameter("out", [S, D], F32, isOutput=True)

    with tile.TileContext(nc) as tc, \
         nc.allow_low_precision(reason="float32r operands feed the PE at full rate"):
        with (tc.tile_pool(name="wp", bufs=1) as wp,
              tc.tile_pool(name="xp", bufs=2) as xp,
              tc.tile_pool(name="rt", bufs=2) as rt,
              tc.tile_pool(name="ptp", bufs=3) as ptp,
              tc.tile_pool(name="rcp", bufs=2) as rcp,
              tc.tile_pool(name="oevp", bufs=2) as oevp,
              tc.tile_pool(name="ps", bufs=1, space="PSUM") as ps_pool,
              tc.tile_pool(name="ot_ps", bufs=2, space="PSUM") as ot_ps):

            # ---- persistent tiles
            wq_s = wp.tile([128, G * NKT * 128], F32R, name="wq_s")
            wk_s = wp.tile([128, G * NKT * 128], F32R, name="wk_s")
            wv_s = wp.tile([128, NKT * E], F32R, name="wv_s")
            wo_s = wp.tile([128, G * D], F32R, name="wo_s")
            cs_s = wp.tile([128, S], F32, name="cs_s")
            sn_s = wp.tile([128, S], F32, name="sn_s")
            triA_s = wp.tile([128, 256], F32R, name="triA_s")
            triB_s = wp.tile([128, 256], F32R, name="triB_s")
            on_s = wp.tile([128, 64], F32R, name="on_s")
            idn_s = wp.tile([128, 128], F32R, name="idn_s")
            qtr = wp.tile([128, G * S], F32R, name="qtr")
            ktr = wp.tile([128, G * S], F32R, name="ktr")
            vv = wp.tile([128, NST * HPC * VW], F32R, name="vv")
            otn = wp.tile([128, G * S], F32R, name="otn")

            # weight/const DMAs, ordered so the first QKV group's operands
            # land first (per-g slices of wq/wk are contiguous)
            HW = NKT * 128  # columns per head-pair in wq/wk
            nc.sync.dma_start(wq_s[:, 0:HW], wq[:, 0:HW])
            nc.sync.dma_start(wk_s[:, 0:HW], wk[:, 0:HW])
            nc.sync.dma_start(cs_s[:, 0:S // 2], cs[:, 0:S // 2])
            nc.sync.dma_start(sn_s[:, 0:S // 2], sn[:, 0:S // 2])
            nc.sync.dma_start(wq_s[:, HW:2 * HW], wq[:, HW:2 * HW])
            nc.sync.dma_start(wk_s[:, HW:2 * HW], wk[:, HW:2 * HW])
            nc.sync.dma_start(wv_s[:], wv[:])
            nc.sync.dma_start(triA_s[:], triA[:])
            nc.sync.dma_start(triB_s[:], triB[:])
            nc.sync.dma_start(idn_s[:], idn[:])
            nc.sync.dma_start(cs_s[:, S // 2:S], cs[:, S // 2:S])
            nc.sync.dma_start(sn_s[:, S // 2:S], sn[:, S // 2:S])
            nc.sync.dma_start(wo_s[:], wo[:])
            # ones columns of V+ (for the softmax denominator)
            vv_r = vv[:].rearrange("p (st h c) -> p st h c", st=NST, h=HPC, c=VW)
            nc.vector.memset(vv_r[:, :, :, HD:HD + 1].bitcast(F32), 1.0)
            nc.vector.memset(on_s[:].bitcast(F32), 1.0)

            def body():
                xtas = {}

                # ================= QKV + RoPE work items =================
                def make_qkv_items(sc, vg_defer=0):
                    def fetch():
                        # one 2MB transfer: ~300 GB/s vs ~90 GB/s for 512KB
                        # slices (per-dma fixed cost dominates small transfers)
                        xta = xp.tile([128, NKT * 512], F32R, tag="x", name="xta")
                        nc.sync.dma_start(
                            xta[:], xT[:, sc * NKT * 512:(sc + 1) * NKT * 512])
                        xtas[sc] = xta

                    def qk(w_s, g, dst):
                        def f():
                            xta = xtas[sc]
                            if phase < 0:
                                return
                            ps = ps_pool.tile([128, 512], F32, tag="mm", bufs=2, name="ps")
                            for kt in range(NKT):
                                nc.tensor.matmul(
                                    ps[:], w_s[:, (g * NKT + kt) * 128:(g * NKT + kt + 1) * 128],
                                    xta[:, kt * 512:(kt + 1) * 512],
                                    start=(kt == 0), stop=(kt == NKT - 1))
                            if phase == 0:
                                return
                            # RoPE: rot = ps*cos + swap(ps)*sgn_sin
                            qsw = rt.tile([128, 512], F32, tag="qsw", name="qsw")
                            m1 = rt.tile([128, 512], F32, tag="m1", name="m1")
                            m2 = rt.tile([128, 512], F32, tag="m2", name="m2")
                            nc.vector.stream_shuffle(qsw[:], ps[:], SWAP_MASK)
                            nc.vector.tensor_tensor(m1[:], ps[:], cs_s[:, sc * 512:(sc + 1) * 512], OP.mult)
                            nc.gpsimd.tensor_tensor(m2[:], qsw[:], sn_s[:, sc * 512:(sc + 1) * 512], OP.mult)
                            nc.gpsimd.tensor_tensor(
                                dst[:, g * S + sc * 512: g * S + (sc + 1) * 512], m1[:], m2[:], OP.add)
                        return f

                    def vg(stl):
                        def f():
                            xta = xtas[sc]
                            if phase < 0:
                                return
                            st = sc * 4 + stl
                            ps = ps_pool.tile([128, 512], F32, tag="mm", bufs=2, name="psv")
                            for kt in range(NKT):
                                nc.tensor.matmul(
                                    ps[:, 0:E], xta[:, kt * 512 + stl * 128: kt * 512 + (stl + 1) * 128],
                                    wv_s[:, kt * E:(kt + 1) * E], start=(kt == 0), stop=(kt == NKT - 1))
                            if phase == 0:
                                return
                            nc.vector.tensor_copy(
                                vv_r[:, st, :, 0:HD], ps[:, 0:E].rearrange("p (h c) -> p h c", h=HPC, c=HD))
                        return f

                    items = [fetch,
                             qk(wq_s, 0, qtr), qk(wk_s, 0, ktr),
                             qk(wq_s, 1, qtr), qk(wk_s, 1, ktr)]
                    items += [vg(stl) for stl in range(4)]
                    if vg_defer:
                        return items[:-vg_defer], items[-vg_defer:]
                    return items, []

                # ================= attention work items =================
                # Per (q-chunk, head-pair): the two heads' K=64 score matmuls
                # sit in different PE row groups (partition bases 0/64) and
                # overlap in the array.  k-tiles are processed two at a time
                # so one exp covers a [128,1024] two-bank PSUM tile.  PV for
                # a pair is emitted one iteration later so its exp is hidden
                # behind the next pair's score matmuls.
                def cstart_of(kt, qc):
                    coff = max(0, 128 * kt - 512 * qc)
                    return min(coff, 256), coff

                def make_attn_g_items(qc, g):
                    nkt_q = 4 * qc + 4
                    st8 = {}
                    prev = [None]
                    pairs = list(range(0, nkt_q, 2))

                    def emit_pv():
                        ptA, ptB, kts = prev[0]
                        ots = st8["ot"]
                        for j, kt in enumerate(kts):
                            cst, _ = cstart_of(kt, qc)
                            for hh, pt, ot in ((2 * g, ptA, ots[0]), (2 * g + 1, ptB, ots[1])):
                                nc.tensor.matmul(
                                    ot[0:VW, cst:512],
                                    vv[:, kt * HPC * VW + hh * VW: kt * HPC * VW + (hh + 1) * VW],
                                    pt[:, j * 512 + cst: (j + 1) * 512],
                                    start=(kt == 0), stop=(kt == nkt_q - 1))

                    def mk_pair(kt2, first):
                        def f():
                            if first:
                                st8["ot"] = (ot_ps.tile([128, 512], F32, tag="ot", name="otA"),
                                             ot_ps.tile([128, 512], F32, tag="ot", name="otB"))
                            kts = [kt2] + ([kt2 + 1] if kt2 + 1 < nkt_q else [])
                            sA = ps_pool.tile([128, 1024], F32, tag="sc2", bufs=2, name="sA")
                            sB = ps_pool.tile([128, 1024], F32, tag="sc2", bufs=2, name="sB")
                            for j, kt in enumerate(kts):
                                cst, coff = cstart_of(kt, qc)
                                diag = kt >= 4 * qc
                                for hp, stile in ((0, sA), (64, sB)):
                                    nc.tensor.matmul(
                                        stile[:, j * 512 + cst: (j + 1) * 512],
                                        ktr[hp:hp + 64, g * S + kt * 128: g * S + (kt + 1) * 128],
                                        qtr[hp:hp + 64, g * S + qc * 512 + cst: g * S + (qc + 1) * 512],
                                        start=True, stop=not diag)
                                    if diag:
                                        # additive causal mask, 256 wide:
                                        # even coff: [tri|0] at coff,
                                        # odd coff: [-400|tri] at coff-128
                                        tri_s, m0 = (triA_s, coff) if coff % 256 == 0 \
                                            else (triB_s, coff - 128)
                                        nc.tensor.matmul(
                                            stile[:, j * 512 + m0: j * 512 + m0 + 256],
                                            idn_s[:], tri_s[:], start=False, stop=True)
                            c0, _ = cstart_of(kts[0], qc)
                            cend = len(kts) * 512
                            ptA = ptp.tile([128, 1024], F32R, tag="pt", name="ptA")
                            ptB = ptp.tile([128, 1024], F32R, tag="pt", name="ptB")
                            nc.scalar.activation(ptA[:, c0:cend], sA[:, c0:cend], AF.Exp, scale=0.125)
                            nc.scalar.activation(ptB[:, c0:cend], sB[:, c0:cend], AF.Exp, scale=0.125)
                            if prev[0] is not None:
                                emit_pv()
                            prev[0] = (ptA, ptB, kts)
                        return f

                    def tail():
                        otA, otB = st8["ot"]
                        emit_pv()
                        if phase < 3:
                            return
                        # normalize by the denominator row: reciprocal of row
                        # 64, broadcast across 64 partitions via a K=1 matmul,
                        # then multiply out of PSUM.
                        rcA = rcp.tile([128, 512], F32R, tag="rcA", name="rcA")
                        rcB = rcp.tile([128, 512], F32R, tag="rcB", name="rcB")
                        nc.vector.reciprocal(rcA[HD:HD + 1, :], otA[HD:HD + 1, :])
                        nc.vector.reciprocal(rcB[HD:HD + 1, :], otB[HD:HD + 1, :])
                        rbA = ps_pool.tile([128, 512], F32, tag="mm", bufs=2, name="rbA")
                        rbB = ps_pool.tile([128, 512], F32, tag="mm", bufs=2, name="rbB")
                        nc.tensor.matmul(rbA[0:HD, :], on_s[HD:HD + 1, 0:HD],
                                         rcA[HD:HD + 1, :], start=True, stop=True)
                        nc.tensor.matmul(rbB[0:HD, :], on_s[HD:HD + 1, 0:HD],
                                         rcB[HD:HD + 1, :], start=True, stop=True)
                        nc.vector.tensor_copy(rcA[0:HD, :], rbA[0:HD, :])
                        nc.vector.tensor_copy(rcB[0:HD, :], rbB[0:HD, :])
                        for hp, ot, rc in ((0, otA, rcA), (64, otB, rcB)):
                            nc.vector.tensor_tensor(
                                otn[hp:hp + 64, g * S + qc * 512: g * S + (qc + 1) * 512],
                                ot[0:HD, :], rc[0:HD, :], OP.mult)

                    items = [mk_pair(kt2, idx == 0) for idx, kt2 in enumerate(pairs)]
                    items.append(tail)
                    return items

                def make_attn_items(qc):
                    if phase < 2:
                        return []
                    items = []
                    for g in range(G):
                        items += make_attn_g_items(qc, g)
                    return items

                # ================= output projection work items =================
                def make_oproj_items(qc):
                    if phase < 4:
                        return []
                    CW = min(512, D)
                    NPAIR = 2 if S >= 512 else 1
                    items = []

                    def mk(st2):
                        def f():
                            oev = oevp.tile([128, NPAIR * D], F32, tag="oev", name="oev")
                            for half in range(NPAIR):
                                st = qc * 4 + st2 * NPAIR + half
                                for nch in range(D // CW):
                                    op = ps_pool.tile([128, 512], F32, tag="mm", bufs=2, name="opps")
                                    for g in range(G):
                                        nc.tensor.matmul(
                                            op[:, 0:CW], otn[:, g * S + st * 128:g * S + (st + 1) * 128],
                                            wo_s[:, g * D + nch * CW: g * D + (nch + 1) * CW],
                                            start=(g == 0), stop=(g == G - 1))
                                    if nch % 2 == 0:
                                        nc.scalar.copy(
                                            oev[:, half * D + nch * CW: half * D + (nch + 1) * CW], op[:, 0:CW])
                                    else:
                                        nc.vector.tensor_copy(
                                            oev[:, half * D + nch * CW: half * D + (nch + 1) * CW], op[:, 0:CW])
                            st0 = qc * 4 + st2 * NPAIR
                            dst = out[st0 * 128:(st0 + NPAIR) * 128, :].rearrange(
                                "(b p) d -> p b d", b=NPAIR)
                            nc.sync.dma_start(dst, oev[:].rearrange("p (b d) -> p b d", b=NPAIR))
                        return f

                    return [mk(st2) for st2 in range(4 // NPAIR)]

                def merge(a, b):
                    out_items = []
                    j = 0
                    for i, x in enumerate(a):
                        out_items.append(x)
                        want = (i + 1) * len(b) // len(a) if a else len(b)
                        while j < want:
                            out_items.append(b[j])
                            j += 1
                    out_items.extend(b[j:])
                    return out_items

                # drive: chunk 0's QKV runs alone; attention for chunk qc is
                # interleaved with QKV for chunk qc+1 and the O-projection
                # for chunk qc-1.  Two V-groups of the last chunk and the
                # last O-projections pad the final (ACT-heaviest) phase.
                sched = []
                it0, _ = make_qkv_items(0)
                sched += it0
                deferred = []
                for sc in range(1, NSC):
                    fill, d2 = make_qkv_items(sc, vg_defer=2 if sc == NSC - 1 else 0)
                    if d2:
                        deferred = d2
                    if sc >= 2:
                        fill = fill + make_oproj_items(sc - 2)
                    sched += merge(make_attn_items(sc - 1), fill)
                # deferred V-groups must land before the PV matmuls that read
                # them (program order defines dataflow): spread them over the
                # first pairs of the last attention phase only.
                last_attn = make_attn_items(NSC - 1)
                npair0 = (4 * (NSC - 1) + 4) // 2   # pairs in g=0
                sched += merge(last_attn[:npair0 - 1], deferred)
                sched += merge(last_attn[npair0 - 1:],
                               make_oproj_items(NSC - 2) if NSC >= 2 else [])
                sched += make_oproj_items(NSC - 1)
                for it in sched:
                    it()

            if use_loop:
                with tc.For_i(0, repeat, 1):
                    body()
            else:
                for _ in range(repeat):
                    body()

    _split_waits(nc)
    return nc


def _rope_tables(S):
    # interleaved dim order: within each 64-partition head block, partition
    # j=2i holds dim i (gets cos, -sin), j=2i+1 holds dim 32+i (cos, +sin)
    inv = 1.0 / (ROPE_BASE ** (np.arange(HALF, dtype=np.float64) / HALF))
    ang = np.arange(S, dtype=np.float64)[:, None] * inv[None, :]  # [S, HALF]
    cos, sin = np.cos(ang), np.sin(ang)
    j = np.arange(128) % HD
    freq = j // 2
    cs = cos[:, freq].T.astype(np.float32)                # [128, S]
    sgn = np.where(j % 2 == 0, -1.0, 1.0)
    sn = (sin[:, freq] * sgn[None, :]).T.astype(np.float32)
    return np.ascontiguousarray(cs), np.ascontiguousarray(sn)


def _tile_rows(a, nt):
    """[nt*128, C] -> [128, nt*C] with block kt at cols [kt*C, (kt+1)*C)."""
    n, c = a.shape
    assert n == nt * 128
    return np.ascontiguousarray(a.reshape(nt, 128, c).transpose(1, 0, 2).reshape(128, nt * c))


def _tile_rows_g(a, nt, g):
    """[nt*128, g*128] -> [128, g*nt*128], head-pair-major: col
    gi*(nt*128) + kt*128 + c = a[kt*128 + p, gi*128 + c]."""
    n, c = a.shape
    assert n == nt * 128 and c == g * 128
    return np.ascontiguousarray(
        a.reshape(nt, 128, g, 128).transpose(1, 2, 0, 3).reshape(128, g * nt * 128))


def _prep_x(x_b, D, S):
    """[S, D] -> [128, S*NKT]: col sc*(NKT*512) + kt*512 + s' = x[sc*512+s', kt*128+p]."""
    NKT, NSC = D // 128, S // 512
    return np.ascontiguousarray(
        x_b.reshape(NSC, 512, NKT, 128).transpose(3, 0, 2, 1).reshape(128, S * NKT))


def _mask_consts():
    # additive causal masks in [k, q] layout (0 where k <= q, else -400;
    # -50 after the 1/8 softmax scale -> exp underflows to ~2e-22).
    # triA = [tri | zeros] applied at even coff; triB = [-400 | tri]
    # applied 128 left of odd coff (the -400 block covers the fully-masked
    # zone that widened PV matmuls read through).
    tri = np.where(np.triu(np.ones((128, 128), dtype=bool)), 0.0, -400.0).astype(np.float32)
    zeros = np.zeros((128, 128), dtype=np.float32)
    neg = np.full((128, 128), -400.0, dtype=np.float32)
    triA = np.ascontiguousarray(np.concatenate([tri, zeros], axis=1))
    triB = np.ascontiguousarray(np.concatenate([neg, tri], axis=1))
    idn = np.eye(128, dtype=np.float32)
    return triA, triB, idn


def _interleave_perm(n_heads):
    """Permutation of head-dim rows: new row 64h+2i <- old 64h+i,
    new row 64h+2i+1 <- old 64h+32+i."""
    perm = np.empty(n_heads * HD, dtype=np.int64)
    for h in range(n_heads):
        base = h * HD
        for i in range(HALF):
            perm[base + 2 * i] = base + i
            perm[base + 2 * i + 1] = base + HALF + i
    return perm


_PROG_CACHE = {}


def kernel(x, Wq, Wk, Wv, Wo):
    B, S, D = x.shape
    H = 16
    HPC = 4                      # heads per core
    GROUPS = H // HPC            # 4 head-groups
    N_CORES = B * GROUPS
    G = HPC // 2

    x = np.asarray(x, dtype=np.float32)
    Wq, Wk, Wv, Wo = (np.asarray(w, dtype=np.float32) for w in (Wq, Wk, Wv, Wo))

    cs, sn = _rope_tables(S)
    triA, triB, idn = _mask_consts()
    NKT = D // 128
    xTs = [_prep_x(x[b], D, S) for b in range(B)]

    perm = _interleave_perm(HPC)
    in_maps = []
    for c in range(N_CORES):
        b, hg = divmod(c, GROUPS)
        e0 = hg * HPC * HD
        e1 = e0 + HPC * HD
        in_maps.append({
            "xT": xTs[b],
            "wq": _tile_rows_g(Wq[e0:e1, :][perm].T, NKT, G),
            "wk": _tile_rows_g(Wk[e0:e1, :][perm].T, NKT, G),
            "wv": _tile_rows(Wv[e0:e1, :].T, NKT),
            "wo": _tile_rows(Wo[:, e0:e1].T, 2),
            "cs": cs, "sn": sn, "triA": triA, "triB": triB, "idn": idn,
        })

    key = (S, D, HPC)
    if key not in _PROG_CACHE:
        _PROG_CACHE[key] = build_program(S, D, HPC)
    nc = _PROG_CACHE[key]
    res = run_bass_kernel_spmd(nc, in_maps, list(range(N_CORES)))

    out = np.zeros((B, S, D), dtype=np.float64)
    for c in range(N_CORES):
        b = c // GROUPS
        out[b] += res.results[c]["out"].astype(np.float64)
    return out.astype(np.float32)


if __name__ == "__main__":
    # mini self-test: one core, small S/D, against a numpy model
    S, D, HPC = 512, 256, 4
    G = HPC // 2
    rng = np.random.default_rng(0)
    x = rng.standard_normal((S, D)).astype(np.float32)
    bound = 1.0 / np.sqrt(D)
    Wq, Wk, Wv = (rng.uniform(-bound, bound, (HPC * HD, D)).astype(np.float32) for _ in range(3))
    Wo = rng.uniform(-bound, bound, (D, HPC * HD)).astype(np.float32)

    # numpy reference (same math as reference.py, restricted to HPC heads)
    q = (x @ Wq.T).reshape(S, HPC, HD).transpose(1, 0, 2)
    k = (x @ Wk.T).reshape(S, HPC, HD).transpose(1, 0, 2)
    v = (x @ Wv.T).reshape(S, HPC, HD).transpose(1, 0, 2)
    inv = 1.0 / (ROPE_BASE ** (np.arange(HALF) / HALF))
    ang = np.arange(S)[:, None] * inv[None, :]
    cosr, sinr = np.cos(ang), np.sin(ang)

    def rope(t):
        t1, t2 = t[..., :HALF], t[..., HALF:]
        return np.concatenate([t1 * cosr - t2 * sinr, t1 * sinr + t2 * cosr], -1)

    q, k = rope(q), rope(k)
    sc_ = np.einsum("hqd,hkd->hqk", q, k) / np.sqrt(HD)
    mask = np.tril(np.ones((S, S), dtype=bool))
    sc_ = np.where(mask, sc_, -np.inf)
    p = np.exp(sc_ - sc_.max(-1, keepdims=True))
    p /= p.sum(-1, keepdims=True)
    ref = np.einsum("hqk,hkd->hqd", p, v).transpose(1, 0, 2).reshape(S, HPC * HD) @ Wo.T

    cs, sn = _rope_tables(S)
    triA, triB, idn = _mask_consts()
    perm = _interleave_perm(HPC)
    in_map = {
        "xT": _prep_x(x, D, S),
        "wq": _tile_rows_g(Wq[perm].T, D // 128, G),
        "wk": _tile_rows_g(Wk[perm].T, D // 128, G),
        "wv": _tile_rows(Wv.T, D // 128),
        "wo": _tile_rows(Wo.T, 2),
        "cs": cs, "sn": sn, "triA": triA, "triB": triB, "idn": idn,
    }
    nc = build_program(S, D, HPC)
    res = run_bass_kernel_spmd(nc, [in_map], [0])
    got = res.results[0]["out"]
    err = np.abs(got - ref)
    rel = err.max() / np.abs(ref).max()
    rms = np.sqrt((err ** 2).mean()) / np.sqrt((ref ** 2).mean())
    print(f"mini: max abs err {err.max():.3e}  max rel {rel:.3e}  rms rel {rms:.3e}")


# revision 10
# speedup vs baseline: 1.2197x; 1.0998x over previous
"""Trainium2 Bass kernel: multi-head causal self-attention with RoPE.

Computes, for x:[B,S,D], Wq/Wk/Wv/Wo:[D,D] (B=2, S=2048, D=1024, H=16 heads,
hd=64):
    q/k/v = (x @ W{q,k,v}.T) -> [B,H,S,hd];  q,k = rope(q), rope(k)
    out   = softmax(causal(q k^T / sqrt(hd))) v   -> merge heads -> @ Wo.T

Sharding: 8 NeuronCores = (2 batches) x (4 head-groups of 4 heads).  Each
core computes its 4 heads' attention plus the partial output projection
(columns of Wo belonging to its heads); the host sums the 4 partial outputs
per batch.

Per-core dataflow (everything in "transposed" space so no PE transposes are
needed):
    xT [D,S] -> QT,KT [hd,S] per head (fp32r matmuls) -> RoPE (DVE shuffle
    + mul/add split across DVE and Pool) -> scoresT[k,q] = KT^T-slice
    matmuls -> exp on ACT (no max-subtraction: |scores/8| <= ~3.2) -> PV
    with a ones-column appended to V so row 64 of the accumulator is the
    softmax denominator -> normalize -> output projection from the
    transposed head outputs.

Scheduling: the emission order software-pipelines the PE stream.  Within
attention for q-chunk qc, PV for kt-pair j is emitted one iteration late
(while exp(j+1) runs on ACT), and QKV matmul groups for chunk qc+1 plus the
O-projection for chunk qc-1 are interleaved between attention pairs so the
PE never sits idle waiting on ACT.  Causal masks are 256-wide fp32r matmuls
(128-wide ones pay a 4x penalty) using two constants: triA=[tri|0] for
even-coff tiles, triB=[-400|tri] for odd-coff tiles; the -400 block also
zeroes (post-exp) the columns that widened PV matmuls read.
"""

import sys

sys.path.insert(0, "/opt/trn_rl_repo")

import numpy as np

import concourse.bass as bass
import concourse.mybir as mybir
import concourse.tile as tile
from concourse.bass_utils import run_bass_kernel_spmd

F32 = mybir.dt.float32
F32R = mybir.dt.float32r
AF = mybir.ActivationFunctionType
OP = mybir.AluOpType

# stream_shuffle's 32-entry mask is a per-quadrant partition permutation
# (applied identically to all four 32-partition quadrants).  We therefore
# store head dims interleaved -- partition 64h+2i holds dim i, 64h+2i+1
# holds dim 32+i -- so the RoPE pair swap is an adjacent-pair exchange.
# The interleave is a shared permutation of Q and K dims (folded into the
# weight slices and rope tables on the host), which leaves q.k scores
# unchanged.
SWAP_MASK = [i ^ 1 for i in range(32)]

HD = 64
HALF = HD // 2
ROPE_BASE = 10000.0


def _split_waits(nc, maxw=1):
    """walrus in this container rejects instructions with more than a couple
    of semaphore waits; hoist excess waits onto preceding NoOps."""
    ctr = 0
    for bb in nc.main_func.blocks:
        insts = bb.instructions
        new = []
        changed = False
        for ins in insts:
            si = ins.sync_info
            if si is not None and si.on_wait and len(si.on_wait) > maxw:
                waits = list(si.on_wait)
                keep, rest = waits[:maxw], waits[maxw:]
                for i in range(0, len(rest), maxw):
                    ctr += 1
                    new.append(mybir.InstNoOp(
                        name=f"WSPLIT-{ctr}", opcode="NoOp", engine=ins.engine,
                        sync_info=mybir.SyncInfo(on_wait=rest[i:i + maxw], on_update=[])))
                si.on_wait = keep
                changed = True
            new.append(ins)
        if changed:
            bb.instructions = new


def build_program(S, D, HPC=4, repeat=1, use_loop=False, phase=4):
    """One-core SPMD program: attention for HPC heads of one batch.

    phase (for perf bisection): -1 = input DMA only, 0 = +QKV matmuls,
    1 = +rope/V-evac, 2 = +scores/exp/PV, 3 = +normalize, 4 = full.
    use_loop wraps `repeat` copies of the body in a tc.For_i (constant
    NEFF size -- used by the loop-slope timing harness).
    """
    NKT = D // 128          # k-tiles over the embedding dim
    NSC = S // 512          # 512-wide s-chunks
    NST = S // 128          # 128-wide s-tiles
    G = HPC // 2            # head pairs
    E = HPC * HD            # per-core head dims
    VW = HD + 1

    nc = bass.Bass()
    # pre-tiled host layouts: xT[p, sc*NKT*512 + kt*512 + s'] = x[sc*512+s', kt*128+p]
    # wq/wk are head-pair-major: col g*(NKT*128) + kt*128 + c
    xT = nc.declare_dram_parameter("xT", [128, S * NKT], F32R, isOutput=False)
    wq = nc.declare_dram_parameter("wq", [128, G * NKT * 128], F32R, isOutput=False)
    wk = nc.declare_dram_parameter("wk", [128, G * NKT * 128], F32R, isOutput=False)
    wv = nc.declare_dram_parameter("wv", [128, NKT * E], F32R, isOutput=False)
    wo = nc.declare_dram_parameter("wo", [128, G * D], F32R, isOutput=False)
    cs = nc.declare_dram_parameter("cs", [128, S], F32, isOutput=False)
    sn = nc.declare_dram_parameter("sn", [128, S], F32, isOutput=False)
    triA = nc.declare_dram_parameter("triA", [128, 256], F32R, isOutput=False)
    triB = nc.declare_dram_parameter("triB", [128, 256], F32R, isOutput=False)
    idn = nc.declare_dram_parameter("idn", [128, 128], F32R, isOutput=False)
    out = nc.declare_dram_parameter("out", [S, D], F32, isOutput=True)

    with tile.TileContext(nc) as tc, \
         nc.allow_low_precision(reason="float32r operands feed the PE at full rate"):
        with (tc.tile_pool(name="wp", bufs=1) as wp,
              tc.tile_pool(name="xp", bufs=2) as xp,
              tc.tile_pool(name="rt", bufs=2) as rt,
              tc.tile_pool(name="ptp", bufs=3) as ptp,
              tc.tile_pool(name="rcp", bufs=2) as rcp,
              tc.tile_pool(name="oevp", bufs=2) as oevp,
              tc.tile_pool(name="ps", bufs=1, space="PSUM") as ps_pool,
              tc.tile_pool(name="ot_ps", bufs=2, space="PSUM") as ot_ps):

            # ---- persistent tiles
            wq_s = wp.tile([128, G * NKT * 128], F32R, name="wq_s")
            wk_s = wp.tile([128, G * NKT * 128], F32R, name="wk_s")
            wv_s = wp.tile([128, NKT * E], F32R, name="wv_s")
            wo_s = wp.tile([128, G * D], F32R, name="wo_s")
            cs_s = wp.tile([128, S], F32, name="cs_s")
            sn_s = wp.tile([128, S], F32, name="sn_s")
            triA_s = wp.tile([128, 256], F32R, name="triA_s")
            triB_s = wp.tile([128, 256], F32R, name="triB_s")
            on_s = wp.tile([128, 64], F32R, name="on_s")
            idn_s = wp.tile([128, 128], F32R, name="idn_s")
            qtr = wp.tile([128, G * S], F32R, name="qtr")
            ktr = wp.tile([128, G * S], F32R, name="ktr")
            vv = wp.tile([128, NST * HPC * VW], F32R, name="vv")
            otn = wp.tile([128, G * S], F32R, name="otn")

            # weight/const DMAs, ordered so the first QKV group's operands
            # land first (per-g slices of wq/wk are contiguous)
            HW = NKT * 128  # columns per head-pair in wq/wk
            nc.sync.dma_start(wq_s[:, 0:HW], wq[:, 0:HW])
            nc.sync.dma_start(wk_s[:, 0:HW], wk[:, 0:HW])
            nc.sync.dma_start(cs_s[:, 0:S // 2], cs[:, 0:S // 2])
            nc.sync.dma_start(sn_s[:, 0:S // 2], sn[:, 0:S // 2])
            nc.sync.dma_start(wq_s[:, HW:2 * HW], wq[:, HW:2 * HW])
            nc.sync.dma_start(wk_s[:, HW:2 * HW], wk[:, HW:2 * HW])
            nc.sync.dma_start(wv_s[:], wv[:])
            nc.sync.dma_start(triA_s[:], triA[:])
            nc.sync.dma_start(triB_s[:], triB[:])
            nc.sync.dma_start(idn_s[:], idn[:])
            nc.sync.dma_start(cs_s[:, S // 2:S], cs[:, S // 2:S])
            nc.sync.dma_start(sn_s[:, S // 2:S], sn[:, S // 2:S])
            nc.sync.dma_start(wo_s[:], wo[:])
            # ones columns of V+ (for the softmax denominator)
            vv_r = vv[:].rearrange("p (st h c) -> p st h c", st=NST, h=HPC, c=VW)
            nc.vector.memset(vv_r[:, :, :, HD:HD + 1].bitcast(F32), 1.0)
            nc.vector.memset(on_s[:].bitcast(F32), 1.0)

            def body():
                xtas = {}

                # ================= QKV + RoPE work items =================
                def make_qkv_items(sc, vg_defer=0):
                    def fetch():
                        # 1MB transfers: large enough to amortize the per-dma
                        # fixed cost, small enough to pipeline with compute
                        xta = xp.tile([128, NKT * 512], F32R, tag="x", name="xta")
                        w = NKT * 256
                        for h in range(2):
                            nc.sync.dma_start(
                                xta[:, h * w:(h + 1) * w],
                                xT[:, sc * NKT * 512 + h * w: sc * NKT * 512 + (h + 1) * w])
                        xtas[sc] = xta

                    def qk(w_s, g, dst):
                        def f():
                            xta = xtas[sc]
                            if phase < 0:
                                return
                            ps = ps_pool.tile([128, 512], F32, tag="mm", bufs=2, name="ps")
                            for kt in range(NKT):
                                nc.tensor.matmul(
                                    ps[:], w_s[:, (g * NKT + kt) * 128:(g * NKT + kt + 1) * 128],
                                    xta[:, kt * 512:(kt + 1) * 512],
                                    start=(kt == 0), stop=(kt == NKT - 1))
                            if phase == 0:
                                return
                            # RoPE: rot = ps*cos + swap(ps)*sgn_sin
                            qsw = rt.tile([128, 512], F32, tag="qsw", name="qsw")
                            m1 = rt.tile([128, 512], F32, tag="m1", name="m1")
                            m2 = rt.tile([128, 512], F32, tag="m2", name="m2")
                            nc.vector.stream_shuffle(qsw[:], ps[:], SWAP_MASK)
                            nc.vector.tensor_tensor(m1[:], ps[:], cs_s[:, sc * 512:(sc + 1) * 512], OP.mult)
                            nc.gpsimd.tensor_tensor(m2[:], qsw[:], sn_s[:, sc * 512:(sc + 1) * 512], OP.mult)
                            nc.gpsimd.tensor_tensor(
                                dst[:, g * S + sc * 512: g * S + (sc + 1) * 512], m1[:], m2[:], OP.add)
                        return f

                    def vg(stl):
                        def f():
                            xta = xtas[sc]
                            if phase < 0:
                                return
                            st = sc * 4 + stl
                            ps = ps_pool.tile([128, 512], F32, tag="mm", bufs=2, name="psv")
                            for kt in range(NKT):
                                nc.tensor.matmul(
                                    ps[:, 0:E], xta[:, kt * 512 + stl * 128: kt * 512 + (stl + 1) * 128],
                                    wv_s[:, kt * E:(kt + 1) * E], start=(kt == 0), stop=(kt == NKT - 1))
                            if phase == 0:
                                return
                            nc.vector.tensor_copy(
                                vv_r[:, st, :, 0:HD], ps[:, 0:E].rearrange("p (h c) -> p h c", h=HPC, c=HD))
                        return f

                    items = [fetch,
                             qk(wq_s, 0, qtr), qk(wk_s, 0, ktr),
                             qk(wq_s, 1, qtr), qk(wk_s, 1, ktr)]
                    items += [vg(stl) for stl in range(4)]
                    if vg_defer:
                        return items[:-vg_defer], items[-vg_defer:]
                    return items, []

                # ================= attention work items =================
                # Per (q-chunk, head-pair): the two heads' K=64 score matmuls
                # sit in different PE row groups (partition bases 0/64) and
                # overlap in the array.  k-tiles are processed two at a time
                # so one exp covers a [128,1024] two-bank PSUM tile.  PV for
                # a pair is emitted one iteration later so its exp is hidden
                # behind the next pair's score matmuls.
                def cstart_of(kt, qc):
                    coff = max(0, 128 * kt - 512 * qc)
                    return min(coff, 256), coff

                def make_attn_g_items(qc, g):
                    nkt_q = 4 * qc + 4
                    st8 = {}
                    prev = [None]
                    pairs = list(range(0, nkt_q, 2))

                    def emit_pv():
                        ptA, ptB, kts = prev[0]
                        ots = st8["ot"]
                        for j, kt in enumerate(kts):
                            cst, _ = cstart_of(kt, qc)
                            for hh, pt, ot in ((2 * g, ptA, ots[0]), (2 * g + 1, ptB, ots[1])):
                                nc.tensor.matmul(
                                    ot[0:VW, cst:512],
                                    vv[:, kt * HPC * VW + hh * VW: kt * HPC * VW + (hh + 1) * VW],
                                    pt[:, j * 512 + cst: (j + 1) * 512],
                                    start=(kt == 0), stop=(kt == nkt_q - 1))

                    def mk_pair(kt2, first):
                        def f():
                            if first:
                                st8["ot"] = (ot_ps.tile([128, 512], F32, tag="ot", name="otA"),
                                             ot_ps.tile([128, 512], F32, tag="ot", name="otB"))
                            kts = [kt2] + ([kt2 + 1] if kt2 + 1 < nkt_q else [])
                            sA = ps_pool.tile([128, 1024], F32, tag="sc2", bufs=2, name="sA")
                            sB = ps_pool.tile([128, 1024], F32, tag="sc2", bufs=2, name="sB")
                            for j, kt in enumerate(kts):
                                cst, coff = cstart_of(kt, qc)
                                diag = kt >= 4 * qc
                                for hp, stile in ((0, sA), (64, sB)):
                                    nc.tensor.matmul(
                                        stile[:, j * 512 + cst: (j + 1) * 512],
                                        ktr[hp:hp + 64, g * S + kt * 128: g * S + (kt + 1) * 128],
                                        qtr[hp:hp + 64, g * S + qc * 512 + cst: g * S + (qc + 1) * 512],
                                        start=True, stop=not diag)
                                    if diag:
                                        # additive causal mask, 256 wide:
                                        # even coff: [tri|0] at coff,
                                        # odd coff: [-400|tri] at coff-128
                                        tri_s, m0 = (triA_s, coff) if coff % 256 == 0 \
                                            else (triB_s, coff - 128)
                                        nc.tensor.matmul(
                                            stile[:, j * 512 + m0: j * 512 + m0 + 256],
                                            idn_s[:], tri_s[:], start=False, stop=True)
                            c0, _ = cstart_of(kts[0], qc)
                            cend = len(kts) * 512
                            ptA = ptp.tile([128, 1024], F32R, tag="pt", name="ptA")
                            ptB = ptp.tile([128, 1024], F32R, tag="pt", name="ptB")
                            nc.scalar.activation(ptA[:, c0:cend], sA[:, c0:cend], AF.Exp, scale=0.125)
                            nc.scalar.activation(ptB[:, c0:cend], sB[:, c0:cend], AF.Exp, scale=0.125)
                            if prev[0] is not None:
                                emit_pv()
                            prev[0] = (ptA, ptB, kts)
                        return f

                    def tail():
                        otA, otB = st8["ot"]
                        emit_pv()
                        if phase < 3:
                            return
                        # normalize by the denominator row: reciprocal of row
                        # 64, broadcast across 64 partitions via a K=1 matmul,
                        # then multiply out of PSUM.
                        rcA = rcp.tile([128, 512], F32R, tag="rcA", name="rcA")
                        rcB = rcp.tile([128, 512], F32R, tag="rcB", name="rcB")
                        nc.vector.reciprocal(rcA[HD:HD + 1, :], otA[HD:HD + 1, :])
                        nc.vector.reciprocal(rcB[HD:HD + 1, :], otB[HD:HD + 1, :])
                        rbA = ps_pool.tile([128, 512], F32, tag="mm", bufs=2, name="rbA")
                        rbB = ps_pool.tile([128, 512], F32, tag="mm", bufs=2, name="rbB")
                        nc.tensor.matmul(rbA[0:HD, :], on_s[HD:HD + 1, 0:HD],
                                         rcA[HD:HD + 1, :], start=True, stop=True)
                        nc.tensor.matmul(rbB[0:HD, :], on_s[HD:HD + 1, 0:HD],
                                         rcB[HD:HD + 1, :], start=True, stop=True)
                        nc.vector.tensor_copy(rcA[0:HD, :], rbA[0:HD, :])
                        nc.vector.tensor_copy(rcB[0:HD, :], rbB[0:HD, :])
                        for hp, ot, rc in ((0, otA, rcA), (64, otB, rcB)):
                            nc.vector.tensor_tensor(
                                otn[hp:hp + 64, g * S + qc * 512: g * S + (qc + 1) * 512],
                                ot[0:HD, :], rc[0:HD, :], OP.mult)

                    items = [mk_pair(kt2, idx == 0) for idx, kt2 in enumerate(pairs)]
                    items.append(tail)
                    return items

                def make_attn_items(qc):
                    if phase < 2:
                        return []
                    items = []
                    for g in range(G):
                        items += make_attn_g_items(qc, g)
                    return items

                # ================= output projection work items =================
                def make_oproj_items(qc):
                    if phase < 4:
                        return []
                    CW = min(512, D)
                    NPAIR = 2 if S >= 512 else 1
                    items = []

                    def mk(st2):
                        def f():
                            oev = oevp.tile([128, NPAIR * D], F32, tag="oev", name="oev")
                            for half in range(NPAIR):
                                st = qc * 4 + st2 * NPAIR + half
                                for nch in range(D // CW):
                                    op = ps_pool.tile([128, 512], F32, tag="mm", bufs=2, name="opps")
                                    for g in range(G):
                                        nc.tensor.matmul(
                                            op[:, 0:CW], otn[:, g * S + st * 128:g * S + (st + 1) * 128],
                                            wo_s[:, g * D + nch * CW: g * D + (nch + 1) * CW],
                                            start=(g == 0), stop=(g == G - 1))
                                    if nch % 2 == 0:
                                        nc.scalar.copy(
                                            oev[:, half * D + nch * CW: half * D + (nch + 1) * CW], op[:, 0:CW])
                                    else:
                                        nc.vector.tensor_copy(
                                            oev[:, half * D + nch * CW: half * D + (nch + 1) * CW], op[:, 0:CW])
                            st0 = qc * 4 + st2 * NPAIR
                            dst = out[st0 * 128:(st0 + NPAIR) * 128, :].rearrange(
                                "(b p) d -> p b d", b=NPAIR)
                            nc.sync.dma_start(dst, oev[:].rearrange("p (b d) -> p b d", b=NPAIR))
                        return f

                    return [mk(st2) for st2 in range(4 // NPAIR)]

                def merge(a, b):
                    out_items = []
                    j = 0
                    for i, x in enumerate(a):
                        out_items.append(x)
                        want = (i + 1) * len(b) // len(a) if a else len(b)
                        while j < want:
                            out_items.append(b[j])
                            j += 1
                    out_items.extend(b[j:])
                    return out_items

                # drive: chunk 0's QKV runs alone; attention for chunk qc is
                # interleaved with QKV for chunk qc+1 and the O-projection
                # for chunk qc-1.  Two V-groups of the last chunk and the
                # last O-projections pad the final (ACT-heaviest) phase.
                sched = []
                it0, _ = make_qkv_items(0)
                sched += it0
                deferred = []
                for sc in range(1, NSC):
                    fill, d2 = make_qkv_items(sc, vg_defer=2 if sc == NSC - 1 else 0)
                    if d2:
                        deferred = d2
                    if sc >= 2:
                        fill = fill + make_oproj_items(sc - 2)
                    sched += merge(make_attn_items(sc - 1), fill)
                # deferred V-groups must land before the PV matmuls that read
                # them (program order defines dataflow): spread them over the
                # first pairs of the last attention phase only.
                last_attn = make_attn_items(NSC - 1)
                npair0 = (4 * (NSC - 1) + 4) // 2   # pairs in g=0
                sched += merge(last_attn[:npair0 - 1], deferred)
                sched += merge(last_attn[npair0 - 1:],
                               make_oproj_items(NSC - 2) if NSC >= 2 else [])
                sched += make_oproj_items(NSC - 1)
                for it in sched:
                    it()

            if use_loop:
                with tc.For_i(0, repeat, 1):
                    body()
            else:
                for _ in range(repeat):
                    body()

    _split_waits(nc)
    return nc


def _rope_tables(S):
    # interleaved dim order: within each 64-partition head block, partition
    # j=2i holds dim i (gets cos, -sin), j=2i+1 holds dim 32+i (cos, +sin)
    inv = 1.0 / (ROPE_BASE ** (np.arange(HALF, dtype=np.float64) / HALF))
    ang = np.arange(S, dtype=np.float64)[:, None] * inv[None, :]  # [S, HALF]
    cos, sin = np.cos(ang), np.sin(ang)
    j = np.arange(128) % HD
    freq = j // 2
    cs = cos[:, freq].T.astype(np.float32)                # [128, S]
    sgn = np.where(j % 2 == 0, -1.0, 1.0)
    sn = (sin[:, freq] * sgn[None, :]).T.astype(np.float32)
    return np.ascontiguousarray(cs), np.ascontiguousarray(sn)


def _tile_rows(a, nt):
    """[nt*128, C] -> [128, nt*C] with block kt at cols [kt*C, (kt+1)*C)."""
    n, c = a.shape
    assert n == nt * 128
    return np.ascontiguousarray(a.reshape(nt, 128, c).transpose(1, 0, 2).reshape(128, nt * c))


def _tile_rows_g(a, nt, g):
    """[nt*128, g*128] -> [128, g*nt*128], head-pair-major: col
    gi*(nt*128) + kt*128 + c = a[kt*128 + p, gi*128 + c]."""
    n, c = a.shape
    assert n == nt * 128 and c == g * 128
    return np.ascontiguousarray(
        a.reshape(nt, 128, g, 128).transpose(1, 2, 0, 3).reshape(128, g * nt * 128))


def _prep_x(x_b, D, S):
    """[S, D] -> [128, S*NKT]: col sc*(NKT*512) + kt*512 + s' = x[sc*512+s', kt*128+p]."""
    NKT, NSC = D // 128, S // 512
    return np.ascontiguousarray(
        x_b.reshape(NSC, 512, NKT, 128).transpose(3, 0, 2, 1).reshape(128, S * NKT))


def _mask_consts():
    # additive causal masks in [k, q] layout (0 where k <= q, else -400;
    # -50 after the 1/8 softmax scale -> exp underflows to ~2e-22).
    # triA = [tri | zeros] applied at even coff; triB = [-400 | tri]
    # applied 128 left of odd coff (the -400 block covers the fully-masked
    # zone that widened PV matmuls read through).
    tri = np.where(np.triu(np.ones((128, 128), dtype=bool)), 0.0, -400.0).astype(np.float32)
    zeros = np.zeros((128, 128), dtype=np.float32)
    neg = np.full((128, 128), -400.0, dtype=np.float32)
    triA = np.ascontiguousarray(np.concatenate([tri, zeros], axis=1))
    triB = np.ascontiguousarray(np.concatenate([neg, tri], axis=1))
    idn = np.eye(128, dtype=np.float32)
    return triA, triB, idn


def _interleave_perm(n_heads):
    """Permutation of head-dim rows: new row 64h+2i <- old 64h+i,
    new row 64h+2i+1 <- old 64h+32+i."""
    perm = np.empty(n_heads * HD, dtype=np.int64)
    for h in range(n_heads):
        base = h * HD
        for i in range(HALF):
            perm[base + 2 * i] = base + i
            perm[base + 2 * i + 1] = base + HALF + i
    return perm


_PROG_CACHE = {}


def kernel(x, Wq, Wk, Wv, Wo):
    B, S, D = x.shape
    H = 16
    HPC = 4                      # heads per core
    GROUPS = H // HPC            # 4 head-groups
    N_CORES = B * GROUPS
    G = HPC // 2

    x = np.asarray(x, dtype=np.float32)
    Wq, Wk, Wv, Wo = (np.asarray(w, dtype=np.float32) for w in (Wq, Wk, Wv, Wo))

    cs, sn = _rope_tables(S)
    triA, triB, idn = _mask_consts()
    NKT = D // 128
    xTs = [_prep_x(x[b], D, S) for b in range(B)]

    perm = _interleave_perm(HPC)
    in_maps = []
    for c in range(N_CORES):
        b, hg = divmod(c, GROUPS)
        e0 = hg * HPC * HD
        e1 = e0 + HPC * HD
        in_maps.append({
            "xT": xTs[b],
            "wq": _tile_rows_g(Wq[e0:e1, :][perm].T, NKT, G),
            "wk": _tile_rows_g(Wk[e0:e1, :][perm].T, NKT, G),
            "wv": _tile_rows(Wv[e0:e1, :].T, NKT),
            "wo": _tile_rows(Wo[:, e0:e1].T, 2),
            "cs": cs, "sn": sn, "triA": triA, "triB": triB, "idn": idn,
        })

    key = (S, D, HPC)
    if key not in _PROG_CACHE:
        _PROG_CACHE[key] = build_program(S, D, HPC)
    nc = _PROG_CACHE[key]
    res = run_bass_kernel_spmd(nc, in_maps, list(range(N_CORES)))

    out = np.zeros((B, S, D), dtype=np.float64)
    for c in range(N_CORES):
        b = c // GROUPS
        out[b] += res.results[c]["out"].astype(np.float64)
    return out.astype(np.float32)


if __name__ == "__main__":
    # mini self-test: one core, small S/D, against a numpy model
    S, D, HPC = 512, 256, 4
    G = HPC // 2
    rng = np.random.default_rng(0)
    x = rng.standard_normal((S, D)).astype(np.float32)
    bound = 1.0 / np.sqrt(D)
    Wq, Wk, Wv = (rng.uniform(-bound, bound, (HPC * HD, D)).astype(np.float32) for _ in range(3))
    Wo = rng.uniform(-bound, bound, (D, HPC * HD)).astype(np.float32)

    # numpy reference (same math as reference.py, restricted to HPC heads)
    q = (x @ Wq.T).reshape(S, HPC, HD).transpose(1, 0, 2)
    k = (x @ Wk.T).reshape(S, HPC, HD).transpose(1, 0, 2)
    v = (x @ Wv.T).reshape(S, HPC, HD).transpose(1, 0, 2)
    inv = 1.0 / (ROPE_BASE ** (np.arange(HALF) / HALF))
    ang = np.arange(S)[:, None] * inv[None, :]
    cosr, sinr = np.cos(ang), np.sin(ang)

    def rope(t):
        t1, t2 = t[..., :HALF], t[..., HALF:]
        return np.concatenate([t1 * cosr - t2 * sinr, t1 * sinr + t2 * cosr], -1)

    q, k = rope(q), rope(k)
    sc_ = np.einsum("hqd,hkd->hqk", q, k) / np.sqrt(HD)
    mask = np.tril(np.ones((S, S), dtype=bool))
    sc_ = np.where(mask, sc_, -np.inf)
    p = np.exp(sc_ - sc_.max(-1, keepdims=True))
    p /= p.sum(-1, keepdims=True)
    ref = np.einsum("hqk,hkd->hqd", p, v).transpose(1, 0, 2).reshape(S, HPC * HD) @ Wo.T

    cs, sn = _rope_tables(S)
    triA, triB, idn = _mask_consts()
    perm = _interleave_perm(HPC)
    in_map = {
        "xT": _prep_x(x, D, S),
        "wq": _tile_rows_g(Wq[perm].T, D // 128, G),
        "wk": _tile_rows_g(Wk[perm].T, D // 128, G),
        "wv": _tile_rows(Wv.T, D // 128),
        "wo": _tile_rows(Wo.T, 2),
        "cs": cs, "sn": sn, "triA": triA, "triB": triB, "idn": idn,
    }
    nc = build_program(S, D, HPC)
    res = run_bass_kernel_spmd(nc, [in_map], [0])
    got = res.results[0]["out"]
    err = np.abs(got - ref)
    rel = err.max() / np.abs(ref).max()
    rms = np.sqrt((err ** 2).mean()) / np.sqrt((ref ** 2).mean())
    print(f"mini: max abs err {err.max():.3e}  max rel {rel:.3e}  rms rel {rms:.3e}")


# revision 19
# speedup vs baseline: 1.6123x; 1.3219x over previous
"""Trainium2 Bass kernel: multi-head causal self-attention with RoPE.

Computes, for x:[B,S,D], Wq/Wk/Wv/Wo:[D,D] (B=2, S=2048, D=1024, H=16 heads,
hd=64):
    q/k/v = (x @ W{q,k,v}.T) -> [B,H,S,hd];  q,k = rope(q), rope(k)
    out   = softmax(causal(q k^T / sqrt(hd))) v   -> merge heads -> @ Wo.T

Sharding: 8 NeuronCores = (2 batches) x (4 head-groups of 4 heads).  Each
core computes its 4 heads' attention plus the partial output projection
(columns of Wo belonging to its heads); the host sums the 4 partial outputs
per batch.

Per-core dataflow (everything in "transposed" space so no PE transposes are
needed):
    xT [D,S] -> QT,KT [hd,S] per head (fp32r matmuls) -> RoPE (DVE shuffle
    + mul/add split across DVE and Pool) -> scoresT[k,q] = KT^T-slice
    matmuls -> exp on ACT (no max-subtraction: |scores/8| <= ~3.2) -> PV
    with a ones-column appended to V so row 64 of the accumulator is the
    softmax denominator -> normalize -> output projection from the
    transposed head outputs.

Scheduling: the emission order software-pipelines the PE stream.  Within
attention for q-chunk qc, PV for kt-pair j is emitted one iteration late
(while exp(j+1) runs on ACT), and QKV matmul groups for chunk qc+1 plus the
O-projection for chunk qc-1 are interleaved between attention pairs so the
PE never sits idle waiting on ACT.  Causal masks are 256-wide fp32r matmuls
(128-wide ones pay a 4x penalty) using two constants: triA=[tri|0] for
even-coff tiles, triB=[-400|tri] for odd-coff tiles; the -400 block also
zeroes (post-exp) the columns that widened PV matmuls read.
"""

import sys

sys.path.insert(0, "/opt/trn_rl_repo")

import numpy as np

import concourse.bass as bass
import concourse.mybir as mybir
import concourse.tile as tile
from concourse.bass_utils import run_bass_kernel_spmd

F32 = mybir.dt.float32
F32R = mybir.dt.float32r
BF16 = mybir.dt.bfloat16
AF = mybir.ActivationFunctionType
OP = mybir.AluOpType

# stream_shuffle's 32-entry mask is a per-quadrant partition permutation
# (applied identically to all four 32-partition quadrants).  We therefore
# store head dims interleaved -- partition 64h+2i holds dim i, 64h+2i+1
# holds dim 32+i -- so the RoPE pair swap is an adjacent-pair exchange.
# The interleave is a shared permutation of Q and K dims (folded into the
# weight slices and rope tables on the host), which leaves q.k scores
# unchanged.
SWAP_MASK = [i ^ 1 for i in range(32)]

HD = 64
HALF = HD // 2
ROPE_BASE = 10000.0


def _split_waits(nc, maxw=1):
    """walrus in this container rejects instructions with more than a couple
    of semaphore waits; hoist excess waits onto preceding NoOps."""
    ctr = 0
    for bb in nc.main_func.blocks:
        insts = bb.instructions
        new = []
        changed = False
        for ins in insts:
            si = ins.sync_info
            if si is not None and si.on_wait and len(si.on_wait) > maxw:
                waits = list(si.on_wait)
                keep, rest = waits[:maxw], waits[maxw:]
                for i in range(0, len(rest), maxw):
                    ctr += 1
                    new.append(mybir.InstNoOp(
                        name=f"WSPLIT-{ctr}", opcode="NoOp", engine=ins.engine,
                        sync_info=mybir.SyncInfo(on_wait=rest[i:i + maxw], on_update=[])))
                si.on_wait = keep
                changed = True
            new.append(ins)
        if changed:
            bb.instructions = new


def build_program(S, D, HPC=4, repeat=1, use_loop=False, phase=4):
    """One-core SPMD program: attention for HPC heads of one batch.

    phase (for perf bisection): -1 = input DMA only, 0 = +QKV matmuls,
    1 = +rope/V-evac, 2 = +scores/exp/PV, 3 = +normalize, 4 = full.
    use_loop wraps `repeat` copies of the body in a tc.For_i (constant
    NEFF size -- used by the loop-slope timing harness).
    """
    NKT = D // 128          # k-tiles over the embedding dim
    NSC = S // 512          # 512-wide s-chunks
    NST = S // 128          # 128-wide s-tiles
    G = HPC // 2            # head pairs
    E = HPC * HD            # per-core head dims
    VW = HD + 1

    nc = bass.Bass()
    # pre-tiled host layouts: xT[p, sc*NKT*512 + kt*512 + s'] = x[sc*512+s', kt*128+p]
    # wq/wk are head-pair-major: col g*(NKT*128) + kt*128 + c
    xT = nc.declare_dram_parameter("xT", [128, S * NKT], BF16, isOutput=False)
    wq = nc.declare_dram_parameter("wq", [128, G * NKT * 128], BF16, isOutput=False)
    wk = nc.declare_dram_parameter("wk", [128, G * NKT * 128], BF16, isOutput=False)
    wv = nc.declare_dram_parameter("wv", [128, NKT * E], BF16, isOutput=False)
    wo = nc.declare_dram_parameter("wo", [128, G * D], BF16, isOutput=False)
    cs = nc.declare_dram_parameter("cs", [128, S], F32, isOutput=False)
    sn = nc.declare_dram_parameter("sn", [128, S], F32, isOutput=False)
    out = nc.declare_dram_parameter("out", [S, D], F32, isOutput=True)

    with tile.TileContext(nc) as tc, \
         nc.allow_low_precision(reason="float32r operands feed the PE at full rate"):
        with (tc.tile_pool(name="wp", bufs=1) as wp,
              tc.tile_pool(name="xp", bufs=2) as xp,
              tc.tile_pool(name="rt", bufs=2) as rt,
              tc.tile_pool(name="ptp", bufs=3) as ptp,
              tc.tile_pool(name="rcp", bufs=2) as rcp,
              tc.tile_pool(name="oevp", bufs=2) as oevp,
              tc.tile_pool(name="ps", bufs=1, space="PSUM") as ps_pool,
              tc.tile_pool(name="ot_ps", bufs=2, space="PSUM") as ot_ps):

            # ---- persistent tiles
            wq_s = wp.tile([128, G * NKT * 128], BF16, name="wq_s")
            wk_s = wp.tile([128, G * NKT * 128], BF16, name="wk_s")
            wv_s = wp.tile([128, NKT * E], BF16, name="wv_s")
            wo_s = wp.tile([128, G * D], BF16, name="wo_s")
            cs_s = wp.tile([128, S], F32, name="cs_s")
            sn_s = wp.tile([128, S], F32, name="sn_s")
            on_s = wp.tile([128, 64], F32R, name="on_s")
            qtr = wp.tile([128, G * S], F32R, name="qtr")
            ktr = wp.tile([128, G * S], F32R, name="ktr")
            vv = wp.tile([128, NST * HPC * VW], BF16, name="vv")
            otn = wp.tile([128, G * S], BF16, name="otn")

            # weight/const DMAs, ordered so the first QKV group's operands
            # land first (per-g slices of wq/wk are contiguous)
            HW = NKT * 128  # columns per head-pair in wq/wk
            nc.sync.dma_start(wq_s[:, 0:HW], wq[:, 0:HW])
            nc.sync.dma_start(wk_s[:, 0:HW], wk[:, 0:HW])
            nc.sync.dma_start(cs_s[:, 0:S // 2], cs[:, 0:S // 2])
            nc.sync.dma_start(sn_s[:, 0:S // 2], sn[:, 0:S // 2])
            nc.sync.dma_start(wq_s[:, HW:2 * HW], wq[:, HW:2 * HW])
            nc.sync.dma_start(wk_s[:, HW:2 * HW], wk[:, HW:2 * HW])
            nc.sync.dma_start(wv_s[:], wv[:])
            nc.sync.dma_start(cs_s[:, S // 2:S], cs[:, S // 2:S])
            nc.sync.dma_start(sn_s[:, S // 2:S], sn[:, S // 2:S])
            nc.sync.dma_start(wo_s[:], wo[:])
            # ones columns of V+ (for the softmax denominator)
            vv_r = vv[:].rearrange("p (st h c) -> p st h c", st=NST, h=HPC, c=VW)
            nc.vector.memset(vv_r[:, :, :, HD:HD + 1], 1.0)
            nc.vector.memset(on_s[:].bitcast(F32), 1.0)

            def body():
                xtas = {}

                # ================= QKV + RoPE work items =================
                def make_qkv_items(sc, vg_defer=0):
                    def fetch():
                        # 512KB slices: first QK matmuls start after slice 0
                        # lands; larger transfers stall the in-order PE stream
                        xta = xp.tile([128, NKT * 512], BF16, tag="x", name="xta")
                        w = NKT * 256
                        for q4 in range(2):
                            nc.sync.dma_start(
                                xta[:, q4 * w:(q4 + 1) * w],
                                xT[:, sc * NKT * 512 + q4 * w: sc * NKT * 512 + (q4 + 1) * w])
                        xtas[sc] = xta

                    def qk(w_s, g, dst):
                        def f():
                            xta = xtas[sc]
                            if phase < 0:
                                return
                            ps = ps_pool.tile([128, 512], F32, tag="mm", bufs=2, name="ps")
                            for kt in range(NKT):
                                nc.tensor.matmul(
                                    ps[:], w_s[:, (g * NKT + kt) * 128:(g * NKT + kt + 1) * 128],
                                    xta[:, kt * 512:(kt + 1) * 512],
                                    start=(kt == 0), stop=(kt == NKT - 1))
                            if phase == 0:
                                return
                            # RoPE: rot = ps*cos + swap(ps)*sgn_sin
                            qsw = rt.tile([128, 512], F32, tag="qsw", name="qsw")
                            m1 = rt.tile([128, 512], F32, tag="m1", name="m1")
                            m2 = rt.tile([128, 512], F32, tag="m2", name="m2")
                            nc.vector.stream_shuffle(qsw[:], ps[:], SWAP_MASK)
                            nc.vector.tensor_tensor(m1[:], ps[:], cs_s[:, sc * 512:(sc + 1) * 512], OP.mult)
                            nc.gpsimd.tensor_tensor(m2[:], qsw[:], sn_s[:, sc * 512:(sc + 1) * 512], OP.mult)
                            nc.gpsimd.tensor_tensor(
                                dst[:, g * S + sc * 512: g * S + (sc + 1) * 512], m1[:], m2[:], OP.add)
                        return f

                    def vg(stl):
                        def f():
                            xta = xtas[sc]
                            if phase < 0:
                                return
                            st = sc * 4 + stl
                            ps = ps_pool.tile([128, 512], F32, tag="mm", bufs=2, name="psv")
                            for kt in range(NKT):
                                nc.tensor.matmul(
                                    ps[:, 0:E], xta[:, kt * 512 + stl * 128: kt * 512 + (stl + 1) * 128],
                                    wv_s[:, kt * E:(kt + 1) * E], start=(kt == 0), stop=(kt == NKT - 1))
                            if phase == 0:
                                return
                            nc.vector.tensor_copy(
                                vv_r[:, st, :, 0:HD], ps[:, 0:E].rearrange("p (h c) -> p h c", h=HPC, c=HD))
                        return f

                    items = [fetch,
                             qk(wq_s, 0, qtr), qk(wk_s, 0, ktr),
                             qk(wq_s, 1, qtr), qk(wk_s, 1, ktr)]
                    items += [vg(stl) for stl in range(4)]
                    if vg_defer:
                        return items[:-vg_defer], items[-vg_defer:]
                    return items, []

                # ================= attention work items =================
                # Per (q-chunk, head-pair): the two heads' K=64 score matmuls
                # sit in different PE row groups (partition bases 0/64) and
                # overlap in the array.  k-tiles are processed two at a time
                # so one exp covers a [128,1024] two-bank PSUM tile.  PV for
                # a pair is emitted one iteration later so its exp is hidden
                # behind the next pair's score matmuls.
                def cstart_of(kt, qc):
                    coff = max(0, 128 * kt - 512 * qc)
                    return min(coff, 256), coff

                def make_attn_g_items(qc, g):
                    nkt_q = 4 * qc + 4
                    st8 = {}
                    prev = [None]
                    pairs = list(range(0, nkt_q, 2))

                    def emit_pv():
                        ptA, ptB, kts = prev[0]
                        ots = st8["ot"]
                        for j, kt in enumerate(kts):
                            cst, _ = cstart_of(kt, qc)
                            for hh, pt, ot in ((2 * g, ptA, ots[0]), (2 * g + 1, ptB, ots[1])):
                                nc.tensor.matmul(
                                    ot[0:VW, cst:512],
                                    vv[:, kt * HPC * VW + hh * VW: kt * HPC * VW + (hh + 1) * VW],
                                    pt[:, j * 512 + cst: (j + 1) * 512],
                                    start=(kt == 0), stop=(kt == nkt_q - 1))

                    def mk_pair(kt2, first):
                        def f():
                            if first:
                                st8["ot"] = (ot_ps.tile([128, 512], F32, tag="ot", name="otA"),
                                             ot_ps.tile([128, 512], F32, tag="ot", name="otB"))
                            kts = [kt2] + ([kt2 + 1] if kt2 + 1 < nkt_q else [])
                            sA = ps_pool.tile([128, 1024], F32, tag="sc2", bufs=2, name="sA")
                            sB = ps_pool.tile([128, 1024], F32, tag="sc2", bufs=2, name="sB")
                            diag = kt2 >= 4 * qc
                            for j, kt in enumerate(kts):
                                cst, coff = cstart_of(kt, qc)
                                for hp, stile in ((0, sA), (64, sB)):
                                    nc.tensor.matmul(
                                        stile[:, j * 512 + cst: (j + 1) * 512],
                                        ktr[hp:hp + 64, g * S + kt * 128: g * S + (kt + 1) * 128],
                                        qtr[hp:hp + 64, g * S + qc * 512 + cst: g * S + (qc + 1) * 512],
                                        start=True, stop=True)
                            c0, _ = cstart_of(kts[0], qc)
                            cend = len(kts) * 512
                            ptA = ptp.tile([128, 1024], BF16, tag="pt", name="ptA")
                            ptB = ptp.tile([128, 1024], BF16, tag="pt", name="ptB")
                            nc.scalar.activation(ptA[:, c0:cend], sA[:, c0:cend], AF.Exp, scale=0.125)
                            nc.scalar.activation(ptB[:, c0:cend], sB[:, c0:cend], AF.Exp, scale=0.125)
                            if diag:
                                # causal mask on the (idle) Pool engine: zero
                                # pt where k > q.  For the pair {4qc+2m,
                                # 4qc+2m+1} the tile-local predicate is
                                # iota = t - 128m - p with identical params
                                # for both kts (only the AP offset differs);
                                # the widened window also zeroes the columns
                                # that widened PV matmuls read through.
                                m = (kt2 - 4 * qc) // 2
                                wsel = 128 * (m + 1)
                                for pt in (ptA, ptB):
                                    for j in range(len(kts)):
                                        o0 = j * 640 + 128 * m
                                        nc.gpsimd.affine_select(
                                            pt[:, o0:o0 + wsel], pt[:, o0:o0 + wsel],
                                            [[1, wsel]], OP.is_ge, 0.0,
                                            base=-128 * m, channel_multiplier=-1)
                            if prev[0] is not None:
                                emit_pv()
                            prev[0] = (ptA, ptB, kts)
                        return f

                    def tail():
                        otA, otB = st8["ot"]
                        emit_pv()
                        if phase < 3:
                            return
                        # normalize by the denominator row: reciprocal of row
                        # 64, broadcast across 64 partitions via a K=1 matmul,
                        # then multiply out of PSUM.
                        rcA = rcp.tile([128, 512], F32R, tag="rcA", name="rcA")
                        rcB = rcp.tile([128, 512], F32R, tag="rcB", name="rcB")
                        nc.vector.reciprocal(rcA[HD:HD + 1, :], otA[HD:HD + 1, :])
                        nc.vector.reciprocal(rcB[HD:HD + 1, :], otB[HD:HD + 1, :])
                        rbA = ps_pool.tile([128, 512], F32, tag="mm", bufs=2, name="rbA")
                        rbB = ps_pool.tile([128, 512], F32, tag="mm", bufs=2, name="rbB")
                        nc.tensor.matmul(rbA[0:HD, :], on_s[HD:HD + 1, 0:HD],
                                         rcA[HD:HD + 1, :], start=True, stop=True)
                        nc.tensor.matmul(rbB[0:HD, :], on_s[HD:HD + 1, 0:HD],
                                         rcB[HD:HD + 1, :], start=True, stop=True)
                        nc.vector.tensor_copy(rcA[0:HD, :], rbA[0:HD, :])
                        nc.vector.tensor_copy(rcB[0:HD, :], rbB[0:HD, :])
                        for hp, ot, rc in ((0, otA, rcA), (64, otB, rcB)):
                            nc.vector.tensor_tensor(
                                otn[hp:hp + 64, g * S + qc * 512: g * S + (qc + 1) * 512],
                                ot[0:HD, :], rc[0:HD, :], OP.mult)

                    items = [mk_pair(kt2, idx == 0) for idx, kt2 in enumerate(pairs)]
                    items.append(tail)
                    return items

                def make_attn_items(qc):
                    if phase < 2:
                        return []
                    items = []
                    for g in range(G):
                        items += make_attn_g_items(qc, g)
                    return items

                # ================= output projection work items =================
                def make_oproj_items(qc):
                    if phase < 4:
                        return []
                    CW = min(512, D)
                    NPAIR = 2 if S >= 512 else 1
                    items = []

                    def mk(st2):
                        def f():
                            oev = oevp.tile([128, NPAIR * D], F32, tag="oev", name="oev")
                            for half in range(NPAIR):
                                st = qc * 4 + st2 * NPAIR + half
                                for nch in range(D // CW):
                                    op = ps_pool.tile([128, 512], F32, tag="mm", bufs=2, name="opps")
                                    for g in range(G):
                                        nc.tensor.matmul(
                                            op[:, 0:CW], otn[:, g * S + st * 128:g * S + (st + 1) * 128],
                                            wo_s[:, g * D + nch * CW: g * D + (nch + 1) * CW],
                                            start=(g == 0), stop=(g == G - 1))
                                    if nch % 2 == 0:
                                        nc.scalar.copy(
                                            oev[:, half * D + nch * CW: half * D + (nch + 1) * CW], op[:, 0:CW])
                                    else:
                                        nc.vector.tensor_copy(
                                            oev[:, half * D + nch * CW: half * D + (nch + 1) * CW], op[:, 0:CW])
                            st0 = qc * 4 + st2 * NPAIR
                            dst = out[st0 * 128:(st0 + NPAIR) * 128, :].rearrange(
                                "(b p) d -> p b d", b=NPAIR)
                            nc.sync.dma_start(dst, oev[:].rearrange("p (b d) -> p b d", b=NPAIR))
                        return f

                    return [mk(st2) for st2 in range(4 // NPAIR)]

                def merge(a, b):
                    out_items = []
                    j = 0
                    for i, x in enumerate(a):
                        out_items.append(x)
                        want = (i + 1) * len(b) // len(a) if a else len(b)
                        while j < want:
                            out_items.append(b[j])
                            j += 1
                    out_items.extend(b[j:])
                    return out_items

                # drive: chunk 0's QKV runs alone; attention for chunk qc is
                # interleaved with QKV for chunk qc+1 and the O-projection
                # for chunk qc-1.  Two V-groups of the last chunk and the
                # last O-projections pad the final (ACT-heaviest) phase.
                sched = []
                it0, _ = make_qkv_items(0)
                sched += it0
                deferred = []
                for sc in range(1, NSC):
                    fill, d2 = make_qkv_items(sc, vg_defer=2 if sc == NSC - 1 else 0)
                    if d2:
                        deferred = d2
                    if sc >= 2:
                        fill = fill + make_oproj_items(sc - 2)
                    sched += merge(make_attn_items(sc - 1), fill)
                # deferred V-groups must land before the PV matmuls that read
                # them (program order defines dataflow): spread them over the
                # first pairs of the last attention phase only.
                last_attn = make_attn_items(NSC - 1)
                npair0 = (4 * (NSC - 1) + 4) // 2   # pairs in g=0
                sched += merge(last_attn[:npair0 - 1], deferred)
                sched += merge(last_attn[npair0 - 1:],
                               make_oproj_items(NSC - 2) if NSC >= 2 else [])
                sched += make_oproj_items(NSC - 1)
                for it in sched:
                    it()

            if use_loop:
                with tc.For_i(0, repeat, 1):
                    body()
            else:
                for _ in range(repeat):
                    body()

    _split_waits(nc)
    return nc


def _rope_tables(S):
    # interleaved dim order: within each 64-partition head block, partition
    # j=2i holds dim i (gets cos, -sin), j=2i+1 holds dim 32+i (cos, +sin)
    inv = 1.0 / (ROPE_BASE ** (np.arange(HALF, dtype=np.float64) / HALF))
    ang = np.arange(S, dtype=np.float64)[:, None] * inv[None, :]  # [S, HALF]
    cos, sin = np.cos(ang), np.sin(ang)
    j = np.arange(128) % HD
    freq = j // 2
    cs = cos[:, freq].T.astype(np.float32)                # [128, S]
    sgn = np.where(j % 2 == 0, -1.0, 1.0)
    sn = (sin[:, freq] * sgn[None, :]).T.astype(np.float32)
    return np.ascontiguousarray(cs), np.ascontiguousarray(sn)


def _tile_rows(a, nt):
    """[nt*128, C] -> [128, nt*C] with block kt at cols [kt*C, (kt+1)*C)."""
    n, c = a.shape
    assert n == nt * 128
    return np.ascontiguousarray(a.reshape(nt, 128, c).transpose(1, 0, 2).reshape(128, nt * c))


def _tile_rows_g(a, nt, g):
    """[nt*128, g*128] -> [128, g*nt*128], head-pair-major: col
    gi*(nt*128) + kt*128 + c = a[kt*128 + p, gi*128 + c]."""
    n, c = a.shape
    assert n == nt * 128 and c == g * 128
    return np.ascontiguousarray(
        a.reshape(nt, 128, g, 128).transpose(1, 2, 0, 3).reshape(128, g * nt * 128))


def _prep_x(x_b, D, S):
    """[S, D] -> [128, S*NKT]: col sc*(NKT*512) + kt*512 + s' = x[sc*512+s', kt*128+p]."""
    NKT, NSC = D // 128, S // 512
    return np.ascontiguousarray(
        x_b.reshape(NSC, 512, NKT, 128).transpose(3, 0, 2, 1).reshape(128, S * NKT))


def _bf16(a):
    import ml_dtypes
    return np.ascontiguousarray(a.astype(ml_dtypes.bfloat16))


def _interleave_perm(n_heads):
    """Permutation of head-dim rows: new row 64h+2i <- old 64h+i,
    new row 64h+2i+1 <- old 64h+32+i."""
    perm = np.empty(n_heads * HD, dtype=np.int64)
    for h in range(n_heads):
        base = h * HD
        for i in range(HALF):
            perm[base + 2 * i] = base + i
            perm[base + 2 * i + 1] = base + HALF + i
    return perm


_PROG_CACHE = {}


def kernel(x, Wq, Wk, Wv, Wo):
    B, S, D = x.shape
    H = 16
    HPC = 4                      # heads per core
    GROUPS = H // HPC            # 4 head-groups
    N_CORES = B * GROUPS
    G = HPC // 2

    x = np.asarray(x, dtype=np.float32)
    Wq, Wk, Wv, Wo = (np.asarray(w, dtype=np.float32) for w in (Wq, Wk, Wv, Wo))

    cs, sn = _rope_tables(S)
    NKT = D // 128
    xTs = [_bf16(_prep_x(x[b], D, S)) for b in range(B)]

    perm = _interleave_perm(HPC)
    in_maps = []
    for c in range(N_CORES):
        b, hg = divmod(c, GROUPS)
        e0 = hg * HPC * HD
        e1 = e0 + HPC * HD
        in_maps.append({
            "xT": xTs[b],
            "wq": _bf16(_tile_rows_g(Wq[e0:e1, :][perm].T, NKT, G)),
            "wk": _bf16(_tile_rows_g(Wk[e0:e1, :][perm].T, NKT, G)),
            "wv": _bf16(_tile_rows(Wv[e0:e1, :].T, NKT)),
            "wo": _bf16(_tile_rows(Wo[:, e0:e1].T, 2)),
            "cs": cs, "sn": sn,
        })

    key = (S, D, HPC)
    if key not in _PROG_CACHE:
        _PROG_CACHE[key] = build_program(S, D, HPC)
    nc = _PROG_CACHE[key]
    res = run_bass_kernel_spmd(nc, in_maps, list(range(N_CORES)))

    out = np.zeros((B, S, D), dtype=np.float64)
    for c in range(N_CORES):
        b = c // GROUPS
        out[b] += res.results[c]["out"].astype(np.float64)
    return out.astype(np.float32)


if __name__ == "__main__":
    # mini self-test: one core, small S/D, against a numpy model
    S, D, HPC = 512, 256, 4
    G = HPC // 2
    rng = np.random.default_rng(0)
    x = rng.standard_normal((S, D)).astype(np.float32)
    bound = 1.0 / np.sqrt(D)
    Wq, Wk, Wv = (rng.uniform(-bound, bound, (HPC * HD, D)).astype(np.float32) for _ in range(3))
    Wo = rng.uniform(-bound, bound, (D, HPC * HD)).astype(np.float32)

    # numpy reference (same math as reference.py, restricted to HPC heads)
    q = (x @ Wq.T).reshape(S, HPC, HD).transpose(1, 0, 2)
    k = (x @ Wk.T).reshape(S, HPC, HD).transpose(1, 0, 2)
    v = (x @ Wv.T).reshape(S, HPC, HD).transpose(1, 0, 2)
    inv = 1.0 / (ROPE_BASE ** (np.arange(HALF) / HALF))
    ang = np.arange(S)[:, None] * inv[None, :]
    cosr, sinr = np.cos(ang), np.sin(ang)

    def rope(t):
        t1, t2 = t[..., :HALF], t[..., HALF:]
        return np.concatenate([t1 * cosr - t2 * sinr, t1 * sinr + t2 * cosr], -1)

    q, k = rope(q), rope(k)
    sc_ = np.einsum("hqd,hkd->hqk", q, k) / np.sqrt(HD)
    mask = np.tril(np.ones((S, S), dtype=bool))
    sc_ = np.where(mask, sc_, -np.inf)
    p = np.exp(sc_ - sc_.max(-1, keepdims=True))
    p /= p.sum(-1, keepdims=True)
    ref = np.einsum("hqk,hkd->hqd", p, v).transpose(1, 0, 2).reshape(S, HPC * HD) @ Wo.T

    cs, sn = _rope_tables(S)
    perm = _interleave_perm(HPC)
    in_map = {
        "xT": _bf16(_prep_x(x, D, S)),
        "wq": _bf16(_tile_rows_g(Wq[perm].T, D // 128, G)),
        "wk": _bf16(_tile_rows_g(Wk[perm].T, D // 128, G)),
        "wv": _bf16(_tile_rows(Wv.T, D // 128)),
        "wo": _bf16(_tile_rows(Wo.T, 2)),
        "cs": cs, "sn": sn,
    }
    nc = build_program(S, D, HPC)
    res = run_bass_kernel_spmd(nc, [in_map], [0])
    got = res.results[0]["out"]
    err = np.abs(got - ref)
    rel = err.max() / np.abs(ref).max()
    rms = np.sqrt((err ** 2).mean()) / np.sqrt((ref ** 2).mean())
    print(f"mini: max abs err {err.max():.3e}  max rel {rel:.3e}  rms rel {rms:.3e}")


# revision 22
# speedup vs baseline: 2.1278x; 1.3197x over previous
"""Trainium2 Bass kernel: multi-head causal self-attention with RoPE.

Computes, for x:[B,S,D], Wq/Wk/Wv/Wo:[D,D] (B=2, S=2048, D=1024, H=16 heads,
hd=64):
    q/k/v = (x @ W{q,k,v}.T) -> [B,H,S,hd];  q,k = rope(q), rope(k)
    out   = softmax(causal(q k^T / sqrt(hd))) v   -> merge heads -> @ Wo.T

Sharding: 8 NeuronCores = (2 batches) x (4 head-groups of 4 heads).  Each
core computes its 4 heads' attention plus the partial output projection
(columns of Wo belonging to its heads); the host sums the 4 partial outputs
per batch.

Per-core dataflow (everything in "transposed" space so no PE transposes are
needed):
    xT [D,S] -> QT,KT [hd,S] per head (fp32r matmuls) -> RoPE (DVE shuffle
    + mul/add split across DVE and Pool) -> scoresT[k,q] = KT^T-slice
    matmuls -> exp on ACT (no max-subtraction: |scores/8| <= ~3.2) -> PV
    with a ones-column appended to V so row 64 of the accumulator is the
    softmax denominator -> normalize -> output projection from the
    transposed head outputs.

Scheduling: the emission order software-pipelines the PE stream.  Within
attention for q-chunk qc, PV for kt-pair j is emitted one iteration late
(while exp(j+1) runs on ACT), and QKV matmul groups for chunk qc+1 plus the
O-projection for chunk qc-1 are interleaved between attention pairs so the
PE never sits idle waiting on ACT.  Causal masks are 256-wide fp32r matmuls
(128-wide ones pay a 4x penalty) using two constants: triA=[tri|0] for
even-coff tiles, triB=[-400|tri] for odd-coff tiles; the -400 block also
zeroes (post-exp) the columns that widened PV matmuls read.
"""

import sys

sys.path.insert(0, "/opt/trn_rl_repo")

import numpy as np

import concourse.bass as bass
import concourse.mybir as mybir
import concourse.tile as tile
from concourse.bass_utils import run_bass_kernel_spmd

F32 = mybir.dt.float32
F32R = mybir.dt.float32r
BF16 = mybir.dt.bfloat16
AF = mybir.ActivationFunctionType
OP = mybir.AluOpType

# stream_shuffle's 32-entry mask is a per-quadrant partition permutation
# (applied identically to all four 32-partition quadrants).  We therefore
# store head dims interleaved -- partition 64h+2i holds dim i, 64h+2i+1
# holds dim 32+i -- so the RoPE pair swap is an adjacent-pair exchange.
# The interleave is a shared permutation of Q and K dims (folded into the
# weight slices and rope tables on the host), which leaves q.k scores
# unchanged.
SWAP_MASK = [i ^ 1 for i in range(32)]

HD = 64
HALF = HD // 2
ROPE_BASE = 10000.0


def _split_waits(nc, maxw=1):
    """walrus in this container rejects instructions with more than a couple
    of semaphore waits; hoist excess waits onto preceding NoOps."""
    ctr = 0
    for bb in nc.main_func.blocks:
        insts = bb.instructions
        new = []
        changed = False
        for ins in insts:
            si = ins.sync_info
            if si is not None and si.on_wait and len(si.on_wait) > maxw:
                waits = list(si.on_wait)
                keep, rest = waits[:maxw], waits[maxw:]
                for i in range(0, len(rest), maxw):
                    ctr += 1
                    new.append(mybir.InstNoOp(
                        name=f"WSPLIT-{ctr}", opcode="NoOp", engine=ins.engine,
                        sync_info=mybir.SyncInfo(on_wait=rest[i:i + maxw], on_update=[])))
                si.on_wait = keep
                changed = True
            new.append(ins)
        if changed:
            bb.instructions = new


def build_program(S, D, HPC=4, repeat=1, use_loop=False, phase=4, qkv_dt=BF16):
    """One-core SPMD program: attention for HPC heads of one batch.

    phase (for perf bisection): -1 = input DMA only, 0 = +QKV matmuls,
    1 = +rope/V-evac, 2 = +scores/exp/PV, 3 = +normalize, 4 = full.
    use_loop wraps `repeat` copies of the body in a tc.For_i (constant
    NEFF size -- used by the loop-slope timing harness).
    """
    NKT = D // 128          # k-tiles over the embedding dim
    NSC = S // 512          # 512-wide s-chunks
    NST = S // 128          # 128-wide s-tiles
    G = HPC // 2            # head pairs
    E = HPC * HD            # per-core head dims
    VW = HD + 1

    nc = bass.Bass()
    # pre-tiled host layouts: xT[p, sc*NKT*512 + kt*512 + s'] = x[sc*512+s', kt*128+p]
    # wq/wk are head-pair-major: col g*(NKT*128) + kt*128 + c
    xT = nc.declare_dram_parameter("xT", [128, S * NKT], qkv_dt, isOutput=False)
    wq = nc.declare_dram_parameter("wq", [128, G * NKT * 128], qkv_dt, isOutput=False)
    wk = nc.declare_dram_parameter("wk", [128, G * NKT * 128], qkv_dt, isOutput=False)
    wv = nc.declare_dram_parameter("wv", [128, NKT * E], qkv_dt, isOutput=False)
    wo = nc.declare_dram_parameter("wo", [128, G * D], BF16, isOutput=False)
    cs = nc.declare_dram_parameter("cs", [128, S], F32, isOutput=False)
    sn = nc.declare_dram_parameter("sn", [128, S], F32, isOutput=False)
    out = nc.declare_dram_parameter("out", [S, D], F32, isOutput=True)

    with tile.TileContext(nc) as tc, \
         nc.allow_low_precision(reason="float32r operands feed the PE at full rate"):
        with (tc.tile_pool(name="wp", bufs=1) as wp,
              tc.tile_pool(name="xp", bufs=2) as xp,
              tc.tile_pool(name="rt", bufs=2) as rt,
              tc.tile_pool(name="ptp", bufs=3) as ptp,
              tc.tile_pool(name="rcp", bufs=2) as rcp,
              tc.tile_pool(name="oevp", bufs=2) as oevp,
              tc.tile_pool(name="ps", bufs=1, space="PSUM") as ps_pool,
              tc.tile_pool(name="ot_ps", bufs=2, space="PSUM") as ot_ps):

            # ---- persistent tiles
            wq_s = wp.tile([128, G * NKT * 128], qkv_dt, name="wq_s")
            wk_s = wp.tile([128, G * NKT * 128], qkv_dt, name="wk_s")
            wv_s = wp.tile([128, NKT * E], qkv_dt, name="wv_s")
            wo_s = wp.tile([128, G * D], BF16, name="wo_s")
            cs_s = wp.tile([128, S], F32, name="cs_s")
            sn_s = wp.tile([128, S], F32, name="sn_s")
            on_s = wp.tile([128, 64], F32R, name="on_s")
            qtr = wp.tile([128, G * S], F32R, name="qtr")
            ktr = wp.tile([128, G * S], F32R, name="ktr")
            vv = wp.tile([128, NST * HPC * VW], BF16, name="vv")
            otn = wp.tile([128, G * S], BF16, name="otn")

            # weight/const DMAs, ordered so the first QKV group's operands
            # land first (per-g slices of wq/wk are contiguous)
            HW = NKT * 128  # columns per head-pair in wq/wk
            nc.sync.dma_start(wq_s[:, 0:HW], wq[:, 0:HW])
            nc.sync.dma_start(wk_s[:, 0:HW], wk[:, 0:HW])
            nc.sync.dma_start(cs_s[:, 0:S // 2], cs[:, 0:S // 2])
            nc.sync.dma_start(sn_s[:, 0:S // 2], sn[:, 0:S // 2])
            nc.sync.dma_start(wq_s[:, HW:2 * HW], wq[:, HW:2 * HW])
            nc.sync.dma_start(wk_s[:, HW:2 * HW], wk[:, HW:2 * HW])
            nc.sync.dma_start(wv_s[:], wv[:])
            nc.sync.dma_start(cs_s[:, S // 2:S], cs[:, S // 2:S])
            nc.sync.dma_start(sn_s[:, S // 2:S], sn[:, S // 2:S])
            nc.sync.dma_start(wo_s[:], wo[:])
            # ones columns of V+ (for the softmax denominator)
            vv_r = vv[:].rearrange("p (st h c) -> p st h c", st=NST, h=HPC, c=VW)
            nc.vector.memset(vv_r[:, :, :, HD:HD + 1], 1.0)
            nc.vector.memset(on_s[:].bitcast(F32), 1.0)

            def body():
                xtas = {}

                # ================= QKV + RoPE work items =================
                def make_qkv_items(sc, vg_defer=0):
                    def fetch():
                        # 512KB slices: first QK matmuls start after slice 0
                        # lands; larger transfers stall the in-order PE stream
                        xta = xp.tile([128, NKT * 512], qkv_dt, tag="x", name="xta")
                        w = NKT * 128
                        for q4 in range(4):
                            nc.sync.dma_start(
                                xta[:, q4 * w:(q4 + 1) * w],
                                xT[:, sc * NKT * 512 + q4 * w: sc * NKT * 512 + (q4 + 1) * w])
                        xtas[sc] = xta

                    def qk(w_s, g, dst):
                        def f():
                            xta = xtas[sc]
                            if phase < 0:
                                return
                            ps = ps_pool.tile([128, 512], F32, tag="mm", bufs=2, name="ps")
                            for kt in range(NKT):
                                nc.tensor.matmul(
                                    ps[:], w_s[:, (g * NKT + kt) * 128:(g * NKT + kt + 1) * 128],
                                    xta[:, kt * 512:(kt + 1) * 512],
                                    start=(kt == 0), stop=(kt == NKT - 1))
                            if phase == 0:
                                return
                            # RoPE: rot = ps*cos + swap(ps)*sgn_sin
                            qsw = rt.tile([128, 512], F32, tag="qsw", name="qsw")
                            m1 = rt.tile([128, 512], F32, tag="m1", name="m1")
                            m2 = rt.tile([128, 512], F32, tag="m2", name="m2")
                            nc.vector.stream_shuffle(qsw[:], ps[:], SWAP_MASK)
                            nc.vector.tensor_tensor(m1[:], ps[:], cs_s[:, sc * 512:(sc + 1) * 512], OP.mult)
                            nc.gpsimd.tensor_tensor(m2[:], qsw[:], sn_s[:, sc * 512:(sc + 1) * 512], OP.mult)
                            nc.gpsimd.tensor_tensor(
                                dst[:, g * S + sc * 512: g * S + (sc + 1) * 512], m1[:], m2[:], OP.add)
                        return f

                    def vg(stl):
                        def f():
                            xta = xtas[sc]
                            if phase < 0:
                                return
                            st = sc * 4 + stl
                            ps = ps_pool.tile([128, 512], F32, tag="mm", bufs=2, name="psv")
                            for kt in range(NKT):
                                nc.tensor.matmul(
                                    ps[:, 0:E], xta[:, kt * 512 + stl * 128: kt * 512 + (stl + 1) * 128],
                                    wv_s[:, kt * E:(kt + 1) * E], start=(kt == 0), stop=(kt == NKT - 1))
                            if phase == 0:
                                return
                            nc.vector.tensor_copy(
                                vv_r[:, st, :, 0:HD], ps[:, 0:E].rearrange("p (h c) -> p h c", h=HPC, c=HD))
                        return f

                    items = [fetch,
                             qk(wq_s, 0, qtr), qk(wk_s, 0, ktr),
                             qk(wq_s, 1, qtr), qk(wk_s, 1, ktr)]
                    items += [vg(stl) for stl in range(4)]
                    if vg_defer:
                        return items[:-vg_defer], items[-vg_defer:]
                    return items, []

                # ================= attention work items =================
                # Per (q-chunk, head-pair): the two heads' K=64 score matmuls
                # sit in different PE row groups (partition bases 0/64) and
                # overlap in the array.  k-tiles are processed two at a time
                # so one exp covers a [128,1024] two-bank PSUM tile.  PV for
                # a pair is emitted one iteration later so its exp is hidden
                # behind the next pair's score matmuls.
                def cstart_of(kt, qc):
                    coff = max(0, 128 * kt - 512 * qc)
                    return min(coff, 256), coff

                def make_attn_g_items(qc, g):
                    nkt_q = 4 * qc + 4
                    st8 = {}
                    prev = [None]
                    pairs = list(range(0, nkt_q, 2))

                    def emit_pv():
                        ptA, ptB, kts = prev[0]
                        ots = st8["ot"]
                        for j, kt in enumerate(kts):
                            cst, _ = cstart_of(kt, qc)
                            for hh, pt, ot in ((2 * g, ptA, ots[0]), (2 * g + 1, ptB, ots[1])):
                                nc.tensor.matmul(
                                    ot[0:VW, cst:512],
                                    vv[:, kt * HPC * VW + hh * VW: kt * HPC * VW + (hh + 1) * VW],
                                    pt[:, j * 512 + cst: (j + 1) * 512],
                                    start=(kt == 0), stop=(kt == nkt_q - 1))

                    def mk_pair(kt2, first):
                        def f():
                            if first:
                                st8["ot"] = (ot_ps.tile([128, 512], F32, tag="ot", name="otA"),
                                             ot_ps.tile([128, 512], F32, tag="ot", name="otB"))
                            kts = [kt2] + ([kt2 + 1] if kt2 + 1 < nkt_q else [])
                            sA = ps_pool.tile([128, 1024], F32, tag="sc2", bufs=2, name="sA")
                            sB = ps_pool.tile([128, 1024], F32, tag="sc2", bufs=2, name="sB")
                            diag = kt2 >= 4 * qc
                            for j, kt in enumerate(kts):
                                cst, coff = cstart_of(kt, qc)
                                for hp, stile in ((0, sA), (64, sB)):
                                    nc.tensor.matmul(
                                        stile[:, j * 512 + cst: (j + 1) * 512],
                                        ktr[hp:hp + 64, g * S + kt * 128: g * S + (kt + 1) * 128],
                                        qtr[hp:hp + 64, g * S + qc * 512 + cst: g * S + (qc + 1) * 512],
                                        start=True, stop=True)
                            c0, _ = cstart_of(kts[0], qc)
                            cend = len(kts) * 512
                            ptA = ptp.tile([128, 1024], BF16, tag="pt", name="ptA")
                            ptB = ptp.tile([128, 1024], BF16, tag="pt", name="ptB")
                            nc.scalar.activation(ptA[:, c0:cend], sA[:, c0:cend], AF.Exp, scale=0.125)
                            nc.scalar.activation(ptB[:, c0:cend], sB[:, c0:cend], AF.Exp, scale=0.125)
                            if diag:
                                # causal mask on the (idle) Pool engine: zero
                                # pt where k > q.  For the pair {4qc+2m,
                                # 4qc+2m+1} the tile-local predicate is
                                # iota = t - 128m - p with identical params
                                # for both kts (only the AP offset differs);
                                # the widened window also zeroes the columns
                                # that widened PV matmuls read through.
                                m = (kt2 - 4 * qc) // 2
                                wsel = 128 * (m + 1)
                                for pt in (ptA, ptB):
                                    for j in range(len(kts)):
                                        o0 = j * 640 + 128 * m
                                        nc.gpsimd.affine_select(
                                            pt[:, o0:o0 + wsel], pt[:, o0:o0 + wsel],
                                            [[1, wsel]], OP.is_ge, 0.0,
                                            base=-128 * m, channel_multiplier=-1)
                            if prev[0] is not None:
                                emit_pv()
                            prev[0] = (ptA, ptB, kts)
                        return f

                    def tail():
                        otA, otB = st8["ot"]
                        emit_pv()
                        if phase < 3:
                            return
                        # normalize by the denominator row: reciprocal of row
                        # 64, broadcast across 64 partitions via a K=1 matmul,
                        # then multiply out of PSUM.
                        rcA = rcp.tile([128, 512], F32R, tag="rcA", name="rcA")
                        rcB = rcp.tile([128, 512], F32R, tag="rcB", name="rcB")
                        nc.vector.reciprocal(rcA[HD:HD + 1, :], otA[HD:HD + 1, :])
                        nc.vector.reciprocal(rcB[HD:HD + 1, :], otB[HD:HD + 1, :])
                        rbA = ps_pool.tile([128, 512], F32, tag="mm", bufs=2, name="rbA")
                        rbB = ps_pool.tile([128, 512], F32, tag="mm", bufs=2, name="rbB")
                        nc.tensor.matmul(rbA[0:HD, :], on_s[HD:HD + 1, 0:HD],
                                         rcA[HD:HD + 1, :], start=True, stop=True)
                        nc.tensor.matmul(rbB[0:HD, :], on_s[HD:HD + 1, 0:HD],
                                         rcB[HD:HD + 1, :], start=True, stop=True)
                        nc.vector.tensor_copy(rcA[0:HD, :], rbA[0:HD, :])
                        nc.vector.tensor_copy(rcB[0:HD, :], rbB[0:HD, :])
                        for hp, ot, rc in ((0, otA, rcA), (64, otB, rcB)):
                            nc.vector.tensor_tensor(
                                otn[hp:hp + 64, g * S + qc * 512: g * S + (qc + 1) * 512],
                                ot[0:HD, :], rc[0:HD, :], OP.mult)

                    items = [mk_pair(kt2, idx == 0) for idx, kt2 in enumerate(pairs)]
                    items.append(tail)
                    return items

                def make_attn_items(qc):
                    if phase < 2:
                        return []
                    items = []
                    for g in range(G):
                        items += make_attn_g_items(qc, g)
                    return items

                # ================= output projection work items =================
                def make_oproj_items(qc):
                    if phase < 4:
                        return []
                    CW = min(512, D)
                    NPAIR = 2 if S >= 512 else 1
                    items = []

                    def mk(st2):
                        def f():
                            oev = oevp.tile([128, NPAIR * D], F32, tag="oev", name="oev")
                            for half in range(NPAIR):
                                st = qc * 4 + st2 * NPAIR + half
                                for nch in range(D // CW):
                                    op = ps_pool.tile([128, 512], F32, tag="mm", bufs=2, name="opps")
                                    for g in range(G):
                                        nc.tensor.matmul(
                                            op[:, 0:CW], otn[:, g * S + st * 128:g * S + (st + 1) * 128],
                                            wo_s[:, g * D + nch * CW: g * D + (nch + 1) * CW],
                                            start=(g == 0), stop=(g == G - 1))
                                    if nch % 2 == 0:
                                        nc.scalar.copy(
                                            oev[:, half * D + nch * CW: half * D + (nch + 1) * CW], op[:, 0:CW])
                                    else:
                                        nc.vector.tensor_copy(
                                            oev[:, half * D + nch * CW: half * D + (nch + 1) * CW], op[:, 0:CW])
                            st0 = qc * 4 + st2 * NPAIR
                            dst = out[st0 * 128:(st0 + NPAIR) * 128, :].rearrange(
                                "(b p) d -> p b d", b=NPAIR)
                            nc.sync.dma_start(dst, oev[:].rearrange("p (b d) -> p b d", b=NPAIR))
                        return f

                    return [mk(st2) for st2 in range(4 // NPAIR)]

                def merge(a, b):
                    out_items = []
                    j = 0
                    for i, x in enumerate(a):
                        out_items.append(x)
                        want = (i + 1) * len(b) // len(a) if a else len(b)
                        while j < want:
                            out_items.append(b[j])
                            j += 1
                    out_items.extend(b[j:])
                    return out_items

                # drive: chunk 0's QKV runs alone; attention for chunk qc is
                # interleaved with QKV for chunk qc+1 and the O-projection
                # for chunk qc-1.  Two V-groups of the last chunk and the
                # last O-projections pad the final (ACT-heaviest) phase.
                sched = []
                it0, _ = make_qkv_items(0)
                sched += it0
                deferred = []
                for sc in range(1, NSC):
                    fill, d2 = make_qkv_items(sc, vg_defer=2 if sc == NSC - 1 else 0)
                    if d2:
                        deferred = d2
                    if sc >= 2:
                        fill = fill + make_oproj_items(sc - 2)
                    sched += merge(make_attn_items(sc - 1), fill)
                # deferred V-groups must land before the PV matmuls that read
                # them (program order defines dataflow): spread them over the
                # first pairs of the last attention phase only.
                last_attn = make_attn_items(NSC - 1)
                npair0 = (4 * (NSC - 1) + 4) // 2   # pairs in g=0
                sched += merge(last_attn[:npair0 - 1], deferred)
                sched += merge(last_attn[npair0 - 1:],
                               make_oproj_items(NSC - 2) if NSC >= 2 else [])
                sched += make_oproj_items(NSC - 1)
                for it in sched:
                    it()

            if use_loop:
                with tc.For_i(0, repeat, 1):
                    body()
            else:
                for _ in range(repeat):
                    body()

    _split_waits(nc)
    return nc


def _rope_tables(S):
    # interleaved dim order: within each 64-partition head block, partition
    # j=2i holds dim i (gets cos, -sin), j=2i+1 holds dim 32+i (cos, +sin)
    inv = 1.0 / (ROPE_BASE ** (np.arange(HALF, dtype=np.float64) / HALF))
    ang = np.arange(S, dtype=np.float64)[:, None] * inv[None, :]  # [S, HALF]
    cos, sin = np.cos(ang), np.sin(ang)
    j = np.arange(128) % HD
    freq = j // 2
    cs = cos[:, freq].T.astype(np.float32)                # [128, S]
    sgn = np.where(j % 2 == 0, -1.0, 1.0)
    sn = (sin[:, freq] * sgn[None, :]).T.astype(np.float32)
    return np.ascontiguousarray(cs), np.ascontiguousarray(sn)


def _tile_rows(a, nt):
    """[nt*128, C] -> [128, nt*C] with block kt at cols [kt*C, (kt+1)*C)."""
    n, c = a.shape
    assert n == nt * 128
    return np.ascontiguousarray(a.reshape(nt, 128, c).transpose(1, 0, 2).reshape(128, nt * c))


def _tile_rows_g(a, nt, g):
    """[nt*128, g*128] -> [128, g*nt*128], head-pair-major: col
    gi*(nt*128) + kt*128 + c = a[kt*128 + p, gi*128 + c]."""
    n, c = a.shape
    assert n == nt * 128 and c == g * 128
    return np.ascontiguousarray(
        a.reshape(nt, 128, g, 128).transpose(1, 2, 0, 3).reshape(128, g * nt * 128))


def _prep_x(x_b, D, S):
    """[S, D] -> [128, S*NKT]: col sc*(NKT*512) + kt*512 + s' = x[sc*512+s', kt*128+p]."""
    NKT, NSC = D // 128, S // 512
    return np.ascontiguousarray(
        x_b.reshape(NSC, 512, NKT, 128).transpose(3, 0, 2, 1).reshape(128, S * NKT))


def _bf16(a):
    import ml_dtypes
    return np.ascontiguousarray(a.astype(ml_dtypes.bfloat16))


def _interleave_perm(n_heads):
    """Permutation of head-dim rows: new row 64h+2i <- old 64h+i,
    new row 64h+2i+1 <- old 64h+32+i."""
    perm = np.empty(n_heads * HD, dtype=np.int64)
    for h in range(n_heads):
        base = h * HD
        for i in range(HALF):
            perm[base + 2 * i] = base + i
            perm[base + 2 * i + 1] = base + HALF + i
    return perm


_PROG_CACHE = {}


def kernel(x, Wq, Wk, Wv, Wo):
    B, S, D = x.shape
    H = 16
    HPC = 4                      # heads per core
    GROUPS = H // HPC            # 4 head-groups
    N_CORES = B * GROUPS
    G = HPC // 2

    x = np.asarray(x, dtype=np.float32)
    Wq, Wk, Wv, Wo = (np.asarray(w, dtype=np.float32) for w in (Wq, Wk, Wv, Wo))

    cs, sn = _rope_tables(S)
    NKT = D // 128
    xTs = [_bf16(_prep_x(x[b], D, S)) for b in range(B)]

    perm = _interleave_perm(HPC)
    in_maps = []
    for c in range(N_CORES):
        b, hg = divmod(c, GROUPS)
        e0 = hg * HPC * HD
        e1 = e0 + HPC * HD
        in_maps.append({
            "xT": xTs[b],
            "wq": _bf16(_tile_rows_g(Wq[e0:e1, :][perm].T, NKT, G)),
            "wk": _bf16(_tile_rows_g(Wk[e0:e1, :][perm].T, NKT, G)),
            "wv": _bf16(_tile_rows(Wv[e0:e1, :].T, NKT)),
            "wo": _bf16(_tile_rows(Wo[:, e0:e1].T, 2)),
            "cs": cs, "sn": sn,
        })

    key = (S, D, HPC)
    if key not in _PROG_CACHE:
        _PROG_CACHE[key] = build_program(S, D, HPC)
    nc = _PROG_CACHE[key]
    res = run_bass_kernel_spmd(nc, in_maps, list(range(N_CORES)))

    out = np.zeros((B, S, D), dtype=np.float64)
    for c in range(N_CORES):
        b = c // GROUPS
        out[b] += res.results[c]["out"].astype(np.float64)
    return out.astype(np.float32)


if __name__ == "__main__":
    # mini self-test: one core, small S/D, against a numpy model
    S, D, HPC = 512, 256, 4
    G = HPC // 2
    rng = np.random.default_rng(0)
    x = rng.standard_normal((S, D)).astype(np.float32)
    bound = 1.0 / np.sqrt(D)
    Wq, Wk, Wv = (rng.uniform(-bound, bound, (HPC * HD, D)).astype(np.float32) for _ in range(3))
    Wo = rng.uniform(-bound, bound, (D, HPC * HD)).astype(np.float32)

    # numpy reference (same math as reference.py, restricted to HPC heads)
    q = (x @ Wq.T).reshape(S, HPC, HD).transpose(1, 0, 2)
    k = (x @ Wk.T).reshape(S, HPC, HD).transpose(1, 0, 2)
    v = (x @ Wv.T).reshape(S, HPC, HD).transpose(1, 0, 2)
    inv = 1.0 / (ROPE_BASE ** (np.arange(HALF) / HALF))
    ang = np.arange(S)[:, None] * inv[None, :]
    cosr, sinr = np.cos(ang), np.sin(ang)

    def rope(t):
        t1, t2 = t[..., :HALF], t[..., HALF:]
        return np.concatenate([t1 * cosr - t2 * sinr, t1 * sinr + t2 * cosr], -1)

    q, k = rope(q), rope(k)
    sc_ = np.einsum("hqd,hkd->hqk", q, k) / np.sqrt(HD)
    mask = np.tril(np.ones((S, S), dtype=bool))
    sc_ = np.where(mask, sc_, -np.inf)
    p = np.exp(sc_ - sc_.max(-1, keepdims=True))
    p /= p.sum(-1, keepdims=True)
    ref = np.einsum("hqk,hkd->hqd", p, v).transpose(1, 0, 2).reshape(S, HPC * HD) @ Wo.T

    cs, sn = _rope_tables(S)
    perm = _interleave_perm(HPC)
    in_map = {
        "xT": _bf16(_prep_x(x, D, S)),
        "wq": _bf16(_tile_rows_g(Wq[perm].T, D // 128, G)),
        "wk": _bf16(_tile_rows_g(Wk[perm].T, D // 128, G)),
        "wv": _bf16(_tile_rows(Wv.T, D // 128)),
        "wo": _bf16(_tile_rows(Wo.T, 2)),
        "cs": cs, "sn": sn,
    }
    nc = build_program(S, D, HPC)
    res = run_bass_kernel_spmd(nc, [in_map], [0])
    got = res.results[0]["out"]
    err = np.abs(got - ref)
    rel = err.max() / np.abs(ref).max()
    rms = np.sqrt((err ** 2).mean()) / np.sqrt((ref ** 2).mean())
    print(f"mini: max abs err {err.max():.3e}  max rel {rel:.3e}  rms rel {rms:.3e}")
